# revision 35
# baseline (speedup 1.0000x reference)
"""NTM addressing head (nn_HeadBase) Trainium2 Bass kernel.

Full-input contract: kernel(**inputs) takes the unsharded [256, ...] arrays,
shards batch-dim across 8 NeuronCores (pure data parallel), runs one SPMD Bass
program per core, and gathers the full [256, 4096] output.

Per-core layout (B=32 batches, N=4096, M=64):
  memory[b] is streamed as [128, CB*2048] SBUF tiles (CB=4 batches/chunk)
  with n = p*32 + r (partition p, free = (b, r, m)); 8 KB contiguous per
  partition per batch.

  Phase A per chunk: the mem*k multiply is split GpSimd (2.5 batches) / DVE
  (1.5 batches), both writing an fp16 product tile; ACT squares mem into an
  fp16 tile.  The m=64 reductions run as fp16 tensor-tensor halving trees
  (64->32->16->8, 2x DVE rate) finished by a fp32-out native reduce (8->1).
  Emission is software-pipelined: chunk c's mults are emitted before chunk
  c-1's trees so the in-order DVE queue reaches the mult (and frees the mem
  buffer for DMA c+2) without waiting behind tree work.
  Raw k is broadcast to all partitions immediately after its DMA; the
  beta/||k|| scale is applied as one extra phase-B multiply instead of
  delaying the broadcast.

  Phase B (all batches fused as [128, 1024] f32 tiles): a = beta/||k|| *
  dot / sqrt(ssq) (Newton-reciprocal on DVE; exact InstReciprocal costs
  6.5us), softmax (no max-subtract: |a|<1), gated interpolation, 3-tap
  circular shift via shifted APs; the +-1 partition carries go through two
  128x128 circular-shift matmuls on the idle TensorEngine.  pow via exp/ln
  with activation-table preloads hidden behind DVE work.  Per-batch scalars
  are broadcast to [128, B] via K=1 ones-matmuls; PSUM evacuation on DVE.
"""

import numpy as np

B_FULL, N, M = 256, 4096, 64
NCORES = 8
B = B_FULL // NCORES   # 32 batches per core
P = 128                # SBUF partitions
R = N // P             # 32 rows per partition; n = p*R + r

_NC_CACHE = {}


def _build_body(nc, out_ap, ins):
    """Emit the kernel IR. ins: dict name->AP of DRAM inputs, out_ap: DRAM out."""
    from contextlib import ExitStack

    import concourse.bass as bass
    import concourse.tile as tile
    from concourse import mybir

    f32 = mybir.dt.float32
    f16 = mybir.dt.float16
    Alu = mybir.AluOpType
    Act = mybir.ActivationFunctionType
    Ax = mybir.AxisListType
    AP = bass.AP

    mem_ap = ins["memory"]   # [B, N, M]
    k_ap = ins["k"]          # [B, M]
    beta_ap = ins["beta"]    # [B, 1]
    pw_ap = ins["prev_w"]    # [B, N]
    g_ap = ins["g"]          # [B, 1]
    s_ap = ins["s"]          # [B, 3]
    gam_ap = ins["gamma"]    # [B, 1]

    def bcast_inner(ap2d, n):
        # [P, C] -> [P, C, n] with 0-stride inner dim
        return AP(ap2d.tensor, ap2d.offset, list(ap2d.ap) + [[0, n]])

    def row1(ap1d):
        # prepend a unit partition dim to a 1-d AP
        return AP(ap1d.tensor, ap1d.offset, [[0, 1]] + list(ap1d.ap))

    with tile.TileContext(nc) as tc, ExitStack() as ctx:
        singles = ctx.enter_context(tc.tile_pool(name="singles", bufs=1))
        mem_pool = ctx.enter_context(tc.tile_pool(name="mem", bufs=3))
        pr_pool = ctx.enter_context(tc.tile_pool(name="pr", bufs=2))
        sq_pool = ctx.enter_context(tc.tile_pool(name="sq", bufs=2))
        h_pool = ctx.enter_context(tc.tile_pool(name="h", bufs=1))
        big = ctx.enter_context(tc.tile_pool(name="big", bufs=1))
        ps = ctx.enter_context(tc.tile_pool(name="ps", bufs=2, space="PSUM"))
        ps_big = ctx.enter_context(tc.tile_pool(name="psbig", bufs=1, space="PSUM"))

        # ---- setup ----
        # Small-input DMAs issued from idle engine sequencers (scalar/gpsimd)
        # so the Sync engine's in-order queue is free to start generating the
        # big memory-chunk descriptors immediately.
        ones_col = singles.tile([P, 1], f32, tag="ones_col")
        nc.vector.memset(ones_col, 1.0)
        ones_row = singles.tile([1, P], f32, tag="ones_row")
        nc.vector.memset(ones_row, 1.0)

        # k borrows a rotating memory-chunk slot (it is dead before chunk 2
        # rotates onto this slot; Tile's WAR tracking enforces the ordering)
        k_host = mem_pool.tile([P, 4 * R * M], f32, tag="mt")
        k_row = k_host[0:1, 0 : B * M]
        nc.scalar.dma_start(out=k_row, in_=row1(k_ap.rearrange("b m -> (b m)")))
        b_row = singles.tile([1, B], f32, tag="b_row")
        nc.gpsimd.dma_start(out=b_row, in_=row1(beta_ap.rearrange("b one -> (b one)")))
        g_row = singles.tile([1, B], f32, tag="g_row")
        nc.gpsimd.dma_start(out=g_row, in_=row1(g_ap.rearrange("b one -> (b one)")))
        gm_row = singles.tile([1, B], f32, tag="gm_row")
        nc.gpsimd.dma_start(out=gm_row, in_=row1(gam_ap.rearrange("b one -> (b one)")))
        s_row = singles.tile([1, 3 * B], f32, tag="s_row")
        nc.scalar.dma_start(out=s_row, in_=row1(s_ap.rearrange("b i -> (b i)")))
        # prev_w big tile [P, B*R] in one permuted-AP DMA (128B inner runs)
        pw = big.tile([P, B * R], f32, tag="pw")
        nc.scalar.dma_start(
            out=pw.rearrange("p (b r) -> p b r", r=R),
            in_=pw_ap.rearrange("b (p r) -> p b r", r=R),
        )
        # s_i as [1, B] strided views (stride 3)
        s_perm = s_row.rearrange("p (b i) -> p i b", i=3)
        s_v = [s_perm[:, i, :] for i in range(3)]

        # RAW k broadcast to all partitions ASAP: kb[p, b*M+m] = k[b, m].
        kb_psum = ps_big.tile([P, B * M], f32, tag="kb_psum")
        for j in range(0, B * M, 512):
            nc.tensor.matmul(
                kb_psum[:, j : j + 512], ones_row, k_row[:, j : j + 512],
                start=True, stop=True,
            )
        kb = singles.tile([P, B * M], f32, tag="kb")
        nc.scalar.copy(out=kb, in_=kb_psum)

        # ---- phase A: stream memory in CB-batch chunks ----
        # NOTE: all remaining phase-B setup (bk chain, scalar broadcasts, pwo,
        # shift matrices) is emitted AFTER the last chunk's stream ops so it
        # cannot block the in-order engine queues during phase A.
        CB = 4          # batches per chunk
        NCH = B // CB   # 8 chunks
        GB = 3          # GpSimd multiplies batches [0, GB); DVE the rest
        dot = big.tile([P, B * R], f32, tag="dot")
        ssq = big.tile([P, B * R], f32, tag="ssq")

        def emit_stream(c):
            """DMA + mults + square for chunk c; returns (pr_g, pr_v, sq)."""
            b0 = c * CB
            mt = mem_pool.tile([P, CB * R * M], f32, tag="mt")
            mtb = mt.rearrange("p (b f) -> p b f", b=CB)
            # two half-chunk DMAs for finer arrival granularity
            nc.sync.dma_start(
                out=mtb[:, 0:2],
                in_=mem_ap[b0 : b0 + 2].rearrange("b (p r) m -> p b (r m)", p=P),
            )
            nc.sync.dma_start(
                out=mtb[:, 2:4],
                in_=mem_ap[b0 + 2 : b0 + 4].rearrange(
                    "b (p r) m -> p b (r m)", p=P),
            )
            mt4 = mt.rearrange("p (b r m) -> p b r m", b=CB, m=M)
            kbc = kb[:, b0 * M : (b0 + CB) * M]  # [P, CB*M]
            # SEPARATE product tiles per engine: a tile with writers on two
            # engines gets its writers serialized by the framework, which
            # would put the GpSimd and DVE multiplies in sequence.
            pr_g = pr_pool.tile([P, GB * R * M], f16, tag="pr_g")
            prg4 = pr_g.rearrange("p (b r m) -> p b r m", b=GB, m=M)
            kb_g = AP(kbc.tensor, kbc.offset, [kbc.ap[0], [M, GB], [0, R], [1, M]])
            nc.gpsimd.tensor_tensor(
                out=prg4, in0=mt4[:, 0:GB], in1=kb_g, op=Alu.mult
            )
            pr_v = pr_pool.tile([P, (CB - GB) * R * M], f16, tag="pr_v")
            prv4 = pr_v.rearrange(
                "p (b r m) -> p b r m", b=CB - GB, m=M
            )
            kb_v = AP(
                kbc.tensor, kbc.offset + GB * M,
                [kbc.ap[0], [M, CB - GB], [0, R], [1, M]],
            )
            nc.vector.tensor_tensor(
                out=prv4, in0=mt4[:, GB:CB], in1=kb_v, op=Alu.mult
            )
            # squares on ACT per half-chunk, fp16 out
            sq = sq_pool.tile([P, CB * R * M], f16, tag="sq")
            sqb = sq.rearrange("p (b f) -> p b f", b=CB)
            nc.scalar.square(out=sqb[:, 0:2], in_=mtb[:, 0:2])
            nc.scalar.square(out=sqb[:, 2:4], in_=mtb[:, 2:4])
            return pr_g, pr_v, sq

        def emit_trees(c, pr_g, pr_v, sq):
            """fp16 halving trees (2x DVE) + f32-out final reduce for chunk c."""
            b0 = c * CB
            G = CB * R  # 128 groups of 64
            GG = GB * R
            vs = sq.rearrange("p (g m) -> p g m", m=64)
            vg = pr_g.rearrange("p (g m) -> p g m", m=64)
            vv = pr_v.rearrange("p (g m) -> p g m", m=64)
            for which, dst in (("dot", dot), ("ssq", ssq)):
                h1 = h_pool.tile([P, G * 32], f16, tag="h1")
                h1v = h1.rearrange("p (g m) -> p g m", m=32)
                if which == "dot":
                    nc.vector.tensor_add(
                        out=h1v[:, 0:GG], in0=vg[:, :, 0:32], in1=vg[:, :, 32:64]
                    )
                    nc.vector.tensor_add(
                        out=h1v[:, GG:G], in0=vv[:, :, 0:32], in1=vv[:, :, 32:64]
                    )
                else:
                    nc.vector.tensor_add(
                        out=h1v, in0=vs[:, :, 0:32], in1=vs[:, :, 32:64]
                    )
                h2 = h_pool.tile([P, G * 16], f16, tag="h2")
                h2v = h2.rearrange("p (g m) -> p g m", m=16)
                nc.vector.tensor_add(
                    out=h2v, in0=h1v[:, :, 0:16], in1=h1v[:, :, 16:32]
                )
                h3 = h_pool.tile([P, G * 8], f16, tag="h3")
                h3v = h3.rearrange("p (g m) -> p g m", m=8)
                nc.vector.tensor_add(
                    out=h3v, in0=h2v[:, :, 0:8], in1=h2v[:, :, 8:16]
                )
                nc.vector.tensor_reduce(
                    out=dst[:, b0 * R : (b0 + CB) * R].rearrange(
                        "p (b r) -> p b r", b=CB),
                    in_=h3v.rearrange("p (b r) m -> p (b r) m", b=CB),
                    axis=Ax.X, op=Alu.add,
                )

        st0 = emit_stream(0)

        # ---- phase-B setup, emitted while the engines wait for chunk 0's
        # DMA (~7us of DVE idle): fills the pipeline-fill window for free ----
        # bk = beta / ||k||; k is squared in place (k_row is dead once the
        # kb broadcast matmuls have read it -- Tile WAR tracking orders this)
        nc.vector.tensor_mul(k_row, k_row, k_row)
        ks_row = singles.tile([1, B], f32, tag="ks_row")
        nc.vector.tensor_reduce(
            out=ks_row, in_=k_row.rearrange("p (b m) -> p b m", m=M),
            axis=Ax.X, op=Alu.add,
        )
        # this Sqrt also preloads the table for the big ssq sqrt below
        kn_row = singles.tile([1, B], f32, tag="kn_row")
        nc.scalar.activation(out=kn_row, in_=ks_row, func=Act.Sqrt)
        rk_row = singles.tile([1, B], f32, tag="rk_row")
        nc.vector.reciprocal(out=rk_row, in_=kn_row)
        bk_row = singles.tile([1, B], f32, tag="bk_row")
        nc.vector.tensor_mul(bk_row, b_row, rk_row)
        # omg = 1 - g
        omg_row = singles.tile([1, B], f32, tag="omg_row")
        nc.vector.tensor_scalar(
            out=omg_row, in0=g_row, scalar1=-1.0, scalar2=1.0,
            op0=Alu.mult, op1=Alu.add,
        )
        # broadcast round 1: [bk, omg, s0, s1, s2, gamma] -> [P, 6*B]
        NSC = 6
        asm1 = singles.tile([1, NSC * B], f32, tag="asm1")
        for i, src in enumerate([bk_row, omg_row, s_v[0], s_v[1], s_v[2], gm_row]):
            nc.vector.tensor_copy(asm1[:, i * B : (i + 1) * B], src)
        bc1_ps = ps.tile([P, NSC * B], f32, tag="mm")
        nc.tensor.matmul(bc1_ps, ones_row, asm1, start=True, stop=True)
        BC1 = singles.tile([P, NSC * B], f32, tag="BC1")
        nc.vector.tensor_copy(BC1, bc1_ps)
        BK = BC1[:, 0 * B : 1 * B]
        OMG = BC1[:, 1 * B : 2 * B]
        S0 = BC1[:, 2 * B : 3 * B]
        S1 = BC1[:, 3 * B : 4 * B]
        S2 = BC1[:, 4 * B : 5 * B]
        GAM = BC1[:, 5 * B : 6 * B]
        # pwo = prev_w * (1 - g)
        nc.vector.tensor_mul(
            pw.rearrange("p (b r) -> p b r", r=R),
            pw.rearrange("p (b r) -> p b r", r=R),
            bcast_inner(OMG, R),
        )

        emit_trees(0, *st0)
        for c in range(1, NCH):
            st = emit_stream(c)
            emit_trees(c, *st)

        # circular +-1 partition shift matrices for the conv carries, built on
        # GpSimd AFTER its last mult (the Q7 ISA ops need pipeline drains that
        # would bubble the mult stream if emitted earlier):
        # SD[p, q] = 1 iff q == (p+1) mod P ; SU[p, q] = 1 iff q == (p-1) mod P
        ones_sq = singles.tile([P, P], f32, tag="ones_sq")
        nc.vector.memset(ones_sq, 1.0)
        sd_t = singles.tile([P, P], f32, tag="sd_t")
        nc.gpsimd.affine_select(
            out=sd_t, in_=ones_sq, pattern=[[1, P]], compare_op=Alu.is_equal,
            fill=0.0, base=-1, channel_multiplier=-1,
        )
        SD = singles.tile([P, P], f32, tag="SD")
        nc.gpsimd.affine_select(
            out=SD, in_=sd_t, pattern=[[1, P]], compare_op=Alu.not_equal,
            fill=1.0, base=P - 1, channel_multiplier=-1,
        )
        su_t = singles.tile([P, P], f32, tag="sd_t")
        nc.gpsimd.affine_select(
            out=su_t, in_=ones_sq, pattern=[[1, P]], compare_op=Alu.is_equal,
            fill=0.0, base=1, channel_multiplier=-1,
        )
        SU = singles.tile([P, P], f32, tag="SU")
        nc.gpsimd.affine_select(
            out=SU, in_=su_t, pattern=[[1, P]], compare_op=Alu.not_equal,
            fill=1.0, base=-(P - 1), channel_multiplier=-1,
        )
        # re-preload the Sqrt table after the last square so the phase-B sqrt
        # doesn't pay the switch (the kn sqrt above loaded it too early)
        dummy = singles.tile([1, 1], f32, tag="dummy")
        nc.scalar.activation(out=dummy, in_=ks_row[:, 0:1], func=Act.Sqrt)

        # ---- phase B ----
        def v3(t):
            return t.rearrange("p (b r) -> p b r", r=R)

        # a = (beta/||k||) * dot / sqrt(ssq)
        # rstd lands in ws (overwritten later by the shift); the Newton
        # scratch bitcasts a dead fp16 product tile.
        nc.scalar.activation(out=ssq, in_=ssq, func=Act.Sqrt)
        # preload the Exp table while DVE runs the reciprocal
        nc.scalar.activation(out=dummy, in_=ks_row[:, 0:1], func=Act.Exp)
        ws = big.tile([P, B * R], f32, tag="ws")
        scr16 = sq_pool.tile([P, CB * R * M], f16, tag="sq")
        scr = scr16[:, 0 : 2 * B * R].bitcast(f32)
        nc.vector.reciprocal_approx_accurate(out=ws, in_=ssq, scratch=scr)
        nc.vector.tensor_mul(dot, dot, ws)
        nc.vector.tensor_mul(v3(dot), v3(dot), bcast_inner(BK, R))

        # e = exp(a), in place
        nc.scalar.activation(out=dot, in_=dot, func=Act.Exp)
        e = dot
        # preload the Ln table while DVE runs the softmax/gating chain
        nc.scalar.activation(out=dummy, in_=ks_row[:, 0:1], func=Act.Ln)

        # denom per batch; gd = g/denom
        cs = singles.tile([P, B], f32, tag="cs")
        nc.vector.tensor_reduce(out=cs, in_=v3(e), axis=Ax.X, op=Alu.add)
        den_ps = ps.tile([1, B], f32, tag="mm")
        nc.tensor.matmul(den_ps, ones_col, cs, start=True, stop=True)
        rden_row = singles.tile([1, B], f32, tag="rden_row")
        nc.vector.reciprocal(out=rden_row, in_=den_ps)
        gd_row = singles.tile([1, B], f32, tag="gd_row")
        nc.vector.tensor_mul(gd_row, rden_row, g_row)
        gd_ps = ps.tile([P, B], f32, tag="mm")
        nc.tensor.matmul(gd_ps, ones_row, gd_row, start=True, stop=True)
        GD = singles.tile([P, B], f32, tag="GD")
        nc.vector.tensor_copy(GD, gd_ps)

        # wg = e*gd + pwo   (in place into e)
        nc.vector.tensor_mul(v3(e), v3(e), bcast_inner(GD, R))
        nc.vector.tensor_add(out=e, in0=e, in1=pw)

        # circular 3-tap shift: ws[n] = s1*wg[n] + s0*wg[n-1] + s2*wg[n+1]
        # ta reuses pw's slot (pw died at the wg add); tb reuses ssq's slot
        # (ssq died at the reciprocal)
        ta = big.tile([P, B * R], f32, tag="pw")
        tb = big.tile([P, B * R], f32, tag="ssq")
        wg3, ws3, ta3, tb3 = v3(e), v3(ws), v3(ta), v3(tb)
        nc.vector.tensor_mul(ta3, wg3, bcast_inner(S0, R))
        nc.vector.tensor_mul(tb3, wg3, bcast_inner(S2, R))
        nc.vector.tensor_mul(ws3, wg3, bcast_inner(S1, R))
        # partition carries via circular-shift matmuls on the TensorEngine
        # (issued as soon as ta/tb are ready, overlapping the shifted adds):
        # dn[q, b] = ta[(q-1) mod P, b, R-1];  up[q, b] = tb[(q+1) mod P, b, 0]
        ta_col = AP(ta.tensor, ta.offset + (R - 1), [ta.ap[0], [R, B]])
        tb_col = AP(tb.tensor, tb.offset, [tb.ap[0], [R, B]])
        dn_ps = ps.tile([P, B], f32, tag="mm")
        nc.tensor.matmul(dn_ps, SD, ta_col, start=True, stop=True)
        up_ps = ps.tile([P, B], f32, tag="mm")
        nc.tensor.matmul(up_ps, SU, tb_col, start=True, stop=True)
        nc.vector.tensor_add(
            out=ws3[:, :, 1:R], in0=ws3[:, :, 1:R], in1=ta3[:, :, 0 : R - 1]
        )
        nc.vector.tensor_add(
            out=ws3[:, :, 0 : R - 1], in0=ws3[:, :, 0 : R - 1], in1=tb3[:, :, 1:R]
        )
        nc.vector.tensor_add(
            out=ws3[:, :, 0:1], in0=ws3[:, :, 0:1], in1=bcast_inner(dn_ps, 1)
        )
        nc.vector.tensor_add(
            out=ws3[:, :, R - 1 : R], in0=ws3[:, :, R - 1 : R],
            in1=bcast_inner(up_ps, 1),
        )

        # w_pow = ws ** gamma = exp(gamma * ln(ws))
        nc.scalar.activation(out=ws, in_=ws, func=Act.Ln)
        # preload the Exp table while DVE runs the gamma multiply
        nc.scalar.activation(out=dummy, in_=ks_row[:, 0:1], func=Act.Exp)
        nc.vector.tensor_mul(ws3, ws3, bcast_inner(GAM, R))
        nc.scalar.activation(out=ws, in_=ws, func=Act.Exp)

        # normalize: out = w_pow / (sum + 1e-16)
        cs2 = singles.tile([P, B], f32, tag="cs2")
        nc.vector.tensor_reduce(out=cs2, in_=ws3, axis=Ax.X, op=Alu.add)
        d2_ps = ps.tile([1, B], f32, tag="mm")
        nc.tensor.matmul(d2_ps, ones_col, cs2, start=True, stop=True)
        d2_row = singles.tile([1, B], f32, tag="d2_row")
        nc.vector.tensor_scalar_add(out=d2_row, in0=d2_ps, scalar1=1e-16)
        rd2_row = singles.tile([1, B], f32, tag="rd2_row")
        nc.vector.reciprocal(out=rd2_row, in_=d2_row)
        rd2_ps = ps.tile([P, B], f32, tag="mm")
        nc.tensor.matmul(rd2_ps, ones_row, rd2_row, start=True, stop=True)
        RD2 = singles.tile([P, B], f32, tag="RD2")
        nc.vector.tensor_copy(RD2, rd2_ps)
        nc.vector.tensor_mul(ws3, ws3, bcast_inner(RD2, R))

        nc.sync.dma_start(
            out=out_ap.rearrange("b (p r) -> p b r", r=R),
            in_=ws.rearrange("p (b r) -> p b r", r=R),
        )


def _get_nc():
    if "nc" in _NC_CACHE:
        return _NC_CACHE["nc"]
    from concourse import bacc, mybir

    f32 = mybir.dt.float32
    nc = bacc.Bacc("TRN2", debug=False, num_devices=NCORES)
    ins = {
        "memory": nc.dram_tensor("memory", [B, N, M], f32, kind="ExternalInput").ap(),
        "k": nc.dram_tensor("k", [B, M], f32, kind="ExternalInput").ap(),
        "beta": nc.dram_tensor("beta", [B, 1], f32, kind="ExternalInput").ap(),
        "prev_w": nc.dram_tensor("prev_w", [B, N], f32, kind="ExternalInput").ap(),
        "g": nc.dram_tensor("g", [B, 1], f32, kind="ExternalInput").ap(),
        "s": nc.dram_tensor("s", [B, 3], f32, kind="ExternalInput").ap(),
        "gamma": nc.dram_tensor("gamma", [B, 1], f32, kind="ExternalInput").ap(),
    }
    out_ap = nc.dram_tensor("out", [B, N], f32, kind="ExternalOutput").ap()
    _build_body(nc, out_ap, ins)
    nc.finalize()
    _NC_CACHE["nc"] = nc
    return nc


def _shard_inputs(inputs):
    arrs = {
        name: np.ascontiguousarray(np.asarray(inputs[name], dtype=np.float32))
        for name in ("memory", "k", "beta", "prev_w", "g", "s", "gamma")
    }
    in_maps = []
    for c in range(NCORES):
        sl = slice(c * B, (c + 1) * B)
        in_maps.append({name: np.ascontiguousarray(a[sl]) for name, a in arrs.items()})
    return in_maps


def run(inputs, trace=False):
    from concourse.bass_utils import run_bass_kernel_spmd

    nc = _get_nc()
    in_maps = _shard_inputs(inputs)
    res = run_bass_kernel_spmd(
        nc, in_maps, core_ids=list(range(NCORES)), trace=trace,
        **({"trace_cores": [0]} if trace else {}),
    )
    out = np.concatenate([r["out"] for r in res.results], axis=0)
    return out, res


def kernel(**inputs):
    out, _ = run(inputs, trace=False)
    return out


# revision 38
# speedup vs baseline: 1.0348x; 1.0348x over previous
"""NTM addressing head (nn_HeadBase) Trainium2 Bass kernel.

Full-input contract: kernel(**inputs) takes the unsharded [256, ...] arrays,
shards batch-dim across 8 NeuronCores (pure data parallel), runs one SPMD Bass
program per core, and gathers the full [256, 4096] output.

Per-core layout (B=32 batches, N=4096, M=64):
  memory[b] is streamed as [128, CB*2048] SBUF tiles (CB=4 batches/chunk)
  with n = p*32 + r (partition p, free = (b, r, m)); 8 KB contiguous per
  partition per batch.

  Phase A per chunk: the mem*k multiply is split GpSimd (2.5 batches) / DVE
  (1.5 batches), both writing an fp16 product tile; ACT squares mem into an
  fp16 tile.  The m=64 reductions run as fp16 tensor-tensor halving trees
  (64->32->16->8, 2x DVE rate) finished by a fp32-out native reduce (8->1).
  Emission is software-pipelined: chunk c's mults are emitted before chunk
  c-1's trees so the in-order DVE queue reaches the mult (and frees the mem
  buffer for DMA c+2) without waiting behind tree work.
  Raw k is broadcast to all partitions immediately after its DMA; the
  beta/||k|| scale is applied as one extra phase-B multiply instead of
  delaying the broadcast.

  Phase B (all batches fused as [128, 1024] f32 tiles): a = beta/||k|| *
  dot / sqrt(ssq) (Newton-reciprocal on DVE; exact InstReciprocal costs
  6.5us), softmax (no max-subtract: |a|<1), gated interpolation, 3-tap
  circular shift via shifted APs; the +-1 partition carries go through two
  128x128 circular-shift matmuls on the idle TensorEngine.  pow via exp/ln
  with activation-table preloads hidden behind DVE work.  Per-batch scalars
  are broadcast to [128, B] via K=1 ones-matmuls; PSUM evacuation on DVE.
"""

import numpy as np

B_FULL, N, M = 256, 4096, 64
NCORES = 8
B = B_FULL // NCORES   # 32 batches per core
P = 128                # SBUF partitions
R = N // P             # 32 rows per partition; n = p*R + r

_NC_CACHE = {}


def _build_body(nc, out_ap, ins):
    """Emit the kernel IR. ins: dict name->AP of DRAM inputs, out_ap: DRAM out."""
    from contextlib import ExitStack

    import concourse.bass as bass
    import concourse.tile as tile
    from concourse import mybir

    f32 = mybir.dt.float32
    f16 = mybir.dt.float16
    Alu = mybir.AluOpType
    Act = mybir.ActivationFunctionType
    Ax = mybir.AxisListType
    AP = bass.AP

    mem_ap = ins["memory"]   # [B, N, M]
    k_ap = ins["k"]          # [B, M]
    beta_ap = ins["beta"]    # [B, 1]
    pw_ap = ins["prev_w"]    # [B, N]
    g_ap = ins["g"]          # [B, 1]
    s_ap = ins["s"]          # [B, 3]
    gam_ap = ins["gamma"]    # [B, 1]

    def bcast_inner(ap2d, n):
        # [P, C] -> [P, C, n] with 0-stride inner dim
        return AP(ap2d.tensor, ap2d.offset, list(ap2d.ap) + [[0, n]])

    def row1(ap1d):
        # prepend a unit partition dim to a 1-d AP
        return AP(ap1d.tensor, ap1d.offset, [[0, 1]] + list(ap1d.ap))

    with tile.TileContext(nc) as tc, ExitStack() as ctx:
        singles = ctx.enter_context(tc.tile_pool(name="singles", bufs=1))
        mem_pool = ctx.enter_context(tc.tile_pool(name="mem", bufs=2))
        pr_pool = ctx.enter_context(tc.tile_pool(name="pr", bufs=2))
        sq_pool = ctx.enter_context(tc.tile_pool(name="sq", bufs=2))
        h_pool = ctx.enter_context(tc.tile_pool(name="h", bufs=1))
        big = ctx.enter_context(tc.tile_pool(name="big", bufs=1))
        ps = ctx.enter_context(tc.tile_pool(name="ps", bufs=2, space="PSUM"))
        ps_big = ctx.enter_context(tc.tile_pool(name="psbig", bufs=1, space="PSUM"))

        # ---- setup ----
        # Small-input DMAs issued from idle engine sequencers (scalar/gpsimd)
        # so the Sync engine's in-order queue is free to start generating the
        # big memory-chunk descriptors immediately.
        ones_col = singles.tile([P, 1], f32, tag="ones_col")
        nc.vector.memset(ones_col, 1.0)
        ones_row = singles.tile([1, P], f32, tag="ones_row")
        nc.vector.memset(ones_row, 1.0)

        # k borrows a rotating memory-chunk slot (it is dead before chunk 2
        # rotates onto this slot; Tile's WAR tracking enforces the ordering)
        k_host = mem_pool.tile([P, 4 * R * M], f32, tag="mt")
        k_row = k_host[0:1, 0 : B * M]
        nc.scalar.dma_start(out=k_row, in_=row1(k_ap.rearrange("b m -> (b m)")))
        b_row = singles.tile([1, B], f32, tag="b_row")
        nc.gpsimd.dma_start(out=b_row, in_=row1(beta_ap.rearrange("b one -> (b one)")))
        g_row = singles.tile([1, B], f32, tag="g_row")
        nc.gpsimd.dma_start(out=g_row, in_=row1(g_ap.rearrange("b one -> (b one)")))
        gm_row = singles.tile([1, B], f32, tag="gm_row")
        nc.gpsimd.dma_start(out=gm_row, in_=row1(gam_ap.rearrange("b one -> (b one)")))
        s_row = singles.tile([1, 3 * B], f32, tag="s_row")
        nc.scalar.dma_start(out=s_row, in_=row1(s_ap.rearrange("b i -> (b i)")))
        # prev_w big tile [P, B*R] in one permuted-AP DMA (128B inner runs)
        pw = big.tile([P, B * R], f32, tag="pw")
        nc.scalar.dma_start(
            out=pw.rearrange("p (b r) -> p b r", r=R),
            in_=pw_ap.rearrange("b (p r) -> p b r", r=R),
        )
        # s_i as [1, B] strided views (stride 3)
        s_perm = s_row.rearrange("p (b i) -> p i b", i=3)
        s_v = [s_perm[:, i, :] for i in range(3)]

        # RAW k broadcast to all partitions ASAP: kb[p, b*M+m] = k[b, m].
        kb_psum = ps_big.tile([P, B * M], f32, tag="kb_psum")
        for j in range(0, B * M, 512):
            nc.tensor.matmul(
                kb_psum[:, j : j + 512], ones_row, k_row[:, j : j + 512],
                start=True, stop=True,
            )
        kb = singles.tile([P, B * M], f32, tag="kb")
        nc.scalar.copy(out=kb, in_=kb_psum)

        # ---- phase A: stream memory in CB-batch chunks ----
        # NOTE: all remaining phase-B setup (bk chain, scalar broadcasts, pwo,
        # shift matrices) is emitted AFTER the last chunk's stream ops so it
        # cannot block the in-order engine queues during phase A.
        CB = 4          # batches per chunk
        NCH = B // CB   # 8 chunks
        GB = 2          # GpSimd multiplies batches [0, GB); DVE the rest
        dot = big.tile([P, B * R], f32, tag="dot")
        ssq = big.tile([P, B * R], f32, tag="ssq")

        def emit_stream(c):
            """DMA + mults + square for chunk c; returns (pr_g, pr_v, sq)."""
            b0 = c * CB
            mt = mem_pool.tile([P, CB * R * M], f32, tag="mt")
            mtb = mt.rearrange("p (b f) -> p b f", b=CB)
            # two half-chunk DMAs for finer arrival granularity
            nc.sync.dma_start(
                out=mtb[:, 0:2],
                in_=mem_ap[b0 : b0 + 2].rearrange("b (p r) m -> p b (r m)", p=P),
            )
            nc.sync.dma_start(
                out=mtb[:, 2:4],
                in_=mem_ap[b0 + 2 : b0 + 4].rearrange(
                    "b (p r) m -> p b (r m)", p=P),
            )
            mt4 = mt.rearrange("p (b r m) -> p b r m", b=CB, m=M)
            kbc = kb[:, b0 * M : (b0 + CB) * M]  # [P, CB*M]
            # SEPARATE product tiles per engine, and the GpSimd one stays
            # fp32: a Q7 tensor op with fp16 output stalls the DVE for its
            # whole duration (observed on HW), killing engine overlap.
            pr_g = pr_pool.tile([P, GB * R * M], f32, tag="pr_g")
            prg4 = pr_g.rearrange("p (b r m) -> p b r m", b=GB, m=M)
            kb_g = AP(kbc.tensor, kbc.offset, [kbc.ap[0], [M, GB], [0, R], [1, M]])
            nc.gpsimd.tensor_tensor(
                out=prg4, in0=mt4[:, 0:GB], in1=kb_g, op=Alu.mult
            )
            pr_v = pr_pool.tile([P, (CB - GB) * R * M], f16, tag="pr_v")
            prv4 = pr_v.rearrange(
                "p (b r m) -> p b r m", b=CB - GB, m=M
            )
            kb_v = AP(
                kbc.tensor, kbc.offset + GB * M,
                [kbc.ap[0], [M, CB - GB], [0, R], [1, M]],
            )
            nc.vector.tensor_tensor(
                out=prv4, in0=mt4[:, GB:CB], in1=kb_v, op=Alu.mult
            )
            # squares on ACT per half-chunk, fp16 out
            sq = sq_pool.tile([P, CB * R * M], f16, tag="sq")
            sqb = sq.rearrange("p (b f) -> p b f", b=CB)
            nc.scalar.square(out=sqb[:, 0:2], in_=mtb[:, 0:2])
            nc.scalar.square(out=sqb[:, 2:4], in_=mtb[:, 2:4])
            return pr_g, pr_v, sq

        def emit_trees(c, pr_g, pr_v, sq):
            """fp16 halving trees (2x DVE) + f32-out final reduce for chunk c."""
            b0 = c * CB
            G = CB * R  # 128 groups of 64
            GG = GB * R
            vs = sq.rearrange("p (g m) -> p g m", m=64)
            vg = pr_g.rearrange("p (g m) -> p g m", m=64)
            vv = pr_v.rearrange("p (g m) -> p g m", m=64)
            for which, dst in (("dot", dot), ("ssq", ssq)):
                h1 = h_pool.tile([P, G * 32], f16, tag="h1")
                h1v = h1.rearrange("p (g m) -> p g m", m=32)
                if which == "dot":
                    nc.vector.tensor_add(
                        out=h1v[:, 0:GG], in0=vg[:, :, 0:32], in1=vg[:, :, 32:64]
                    )
                    nc.vector.tensor_add(
                        out=h1v[:, GG:G], in0=vv[:, :, 0:32], in1=vv[:, :, 32:64]
                    )
                else:
                    nc.vector.tensor_add(
                        out=h1v, in0=vs[:, :, 0:32], in1=vs[:, :, 32:64]
                    )
                h2 = h_pool.tile([P, G * 16], f16, tag="h2")
                h2v = h2.rearrange("p (g m) -> p g m", m=16)
                nc.vector.tensor_add(
                    out=h2v, in0=h1v[:, :, 0:16], in1=h1v[:, :, 16:32]
                )
                h3 = h_pool.tile([P, G * 8], f16, tag="h3")
                h3v = h3.rearrange("p (g m) -> p g m", m=8)
                nc.vector.tensor_add(
                    out=h3v, in0=h2v[:, :, 0:8], in1=h2v[:, :, 8:16]
                )
                nc.vector.tensor_reduce(
                    out=dst[:, b0 * R : (b0 + CB) * R].rearrange(
                        "p (b r) -> p b r", b=CB),
                    in_=h3v.rearrange("p (b r) m -> p (b r) m", b=CB),
                    axis=Ax.X, op=Alu.add,
                )

        st0 = emit_stream(0)

        # ---- phase-B setup, emitted while the engines wait for chunk 0's
        # DMA (~7us of DVE idle): fills the pipeline-fill window for free ----
        # bk = beta / ||k||; k is squared in place (k_row is dead once the
        # kb broadcast matmuls have read it -- Tile WAR tracking orders this)
        nc.vector.tensor_mul(k_row, k_row, k_row)
        ks_row = singles.tile([1, B], f32, tag="ks_row")
        nc.vector.tensor_reduce(
            out=ks_row, in_=k_row.rearrange("p (b m) -> p b m", m=M),
            axis=Ax.X, op=Alu.add,
        )
        # this Sqrt also preloads the table for the big ssq sqrt below
        kn_row = singles.tile([1, B], f32, tag="kn_row")
        nc.scalar.activation(out=kn_row, in_=ks_row, func=Act.Sqrt)
        rk_row = singles.tile([1, B], f32, tag="rk_row")
        nc.vector.reciprocal(out=rk_row, in_=kn_row)
        bk_row = singles.tile([1, B], f32, tag="bk_row")
        nc.vector.tensor_mul(bk_row, b_row, rk_row)
        # omg = 1 - g
        omg_row = singles.tile([1, B], f32, tag="omg_row")
        nc.vector.tensor_scalar(
            out=omg_row, in0=g_row, scalar1=-1.0, scalar2=1.0,
            op0=Alu.mult, op1=Alu.add,
        )
        # broadcast round 1: [bk, omg, s0, s1, s2, gamma] -> [P, 6*B]
        NSC = 6
        asm1 = singles.tile([1, NSC * B], f32, tag="asm1")
        for i, src in enumerate([bk_row, omg_row, s_v[0], s_v[1], s_v[2], gm_row]):
            nc.vector.tensor_copy(asm1[:, i * B : (i + 1) * B], src)
        bc1_ps = ps.tile([P, NSC * B], f32, tag="mm")
        nc.tensor.matmul(bc1_ps, ones_row, asm1, start=True, stop=True)
        BC1 = singles.tile([P, NSC * B], f32, tag="BC1")
        nc.vector.tensor_copy(BC1, bc1_ps)
        BK = BC1[:, 0 * B : 1 * B]
        OMG = BC1[:, 1 * B : 2 * B]
        S0 = BC1[:, 2 * B : 3 * B]
        S1 = BC1[:, 3 * B : 4 * B]
        S2 = BC1[:, 4 * B : 5 * B]
        GAM = BC1[:, 5 * B : 6 * B]
        # pwo = prev_w * (1 - g)
        nc.vector.tensor_mul(
            pw.rearrange("p (b r) -> p b r", r=R),
            pw.rearrange("p (b r) -> p b r", r=R),
            bcast_inner(OMG, R),
        )

        emit_trees(0, *st0)
        for c in range(1, NCH):
            st = emit_stream(c)
            emit_trees(c, *st)

        # circular +-1 partition shift matrices for the conv carries, built on
        # GpSimd AFTER its last mult (the Q7 ISA ops need pipeline drains that
        # would bubble the mult stream if emitted earlier):
        # SD[p, q] = 1 iff q == (p+1) mod P ; SU[p, q] = 1 iff q == (p-1) mod P
        ones_sq = singles.tile([P, P], f32, tag="ones_sq")
        nc.vector.memset(ones_sq, 1.0)
        sd_t = singles.tile([P, P], f32, tag="sd_t")
        nc.gpsimd.affine_select(
            out=sd_t, in_=ones_sq, pattern=[[1, P]], compare_op=Alu.is_equal,
            fill=0.0, base=-1, channel_multiplier=-1,
        )
        SD = singles.tile([P, P], f32, tag="SD")
        nc.gpsimd.affine_select(
            out=SD, in_=sd_t, pattern=[[1, P]], compare_op=Alu.not_equal,
            fill=1.0, base=P - 1, channel_multiplier=-1,
        )
        su_t = singles.tile([P, P], f32, tag="sd_t")
        nc.gpsimd.affine_select(
            out=su_t, in_=ones_sq, pattern=[[1, P]], compare_op=Alu.is_equal,
            fill=0.0, base=1, channel_multiplier=-1,
        )
        SU = singles.tile([P, P], f32, tag="SU")
        nc.gpsimd.affine_select(
            out=SU, in_=su_t, pattern=[[1, P]], compare_op=Alu.not_equal,
            fill=1.0, base=-(P - 1), channel_multiplier=-1,
        )
        # re-preload the Sqrt table after the last square so the phase-B sqrt
        # doesn't pay the switch (the kn sqrt above loaded it too early)
        dummy = singles.tile([1, 1], f32, tag="dummy")
        nc.scalar.activation(out=dummy, in_=ks_row[:, 0:1], func=Act.Sqrt)

        # ---- phase B ----
        def v3(t):
            return t.rearrange("p (b r) -> p b r", r=R)

        # a = (beta/||k||) * dot / sqrt(ssq)
        # rstd lands in ws (overwritten later by the shift); the Newton
        # scratch bitcasts a dead fp16 product tile.
        nc.scalar.activation(out=ssq, in_=ssq, func=Act.Sqrt)
        # preload the Exp table while DVE runs the reciprocal
        nc.scalar.activation(out=dummy, in_=ks_row[:, 0:1], func=Act.Exp)
        ws = big.tile([P, B * R], f32, tag="ws")
        scr16 = sq_pool.tile([P, CB * R * M], f16, tag="sq")
        scr = scr16[:, 0 : 2 * B * R].bitcast(f32)
        nc.vector.reciprocal_approx_accurate(out=ws, in_=ssq, scratch=scr)
        nc.vector.tensor_mul(dot, dot, ws)
        nc.vector.tensor_mul(v3(dot), v3(dot), bcast_inner(BK, R))

        # e = exp(a), in place
        nc.scalar.activation(out=dot, in_=dot, func=Act.Exp)
        e = dot
        # preload the Ln table while DVE runs the softmax/gating chain
        nc.scalar.activation(out=dummy, in_=ks_row[:, 0:1], func=Act.Ln)

        # denom per batch; gd = g/denom
        cs = singles.tile([P, B], f32, tag="cs")
        nc.vector.tensor_reduce(out=cs, in_=v3(e), axis=Ax.X, op=Alu.add)
        den_ps = ps.tile([1, B], f32, tag="mm")
        nc.tensor.matmul(den_ps, ones_col, cs, start=True, stop=True)
        rden_row = singles.tile([1, B], f32, tag="rden_row")
        nc.vector.reciprocal(out=rden_row, in_=den_ps)
        gd_row = singles.tile([1, B], f32, tag="gd_row")
        nc.vector.tensor_mul(gd_row, rden_row, g_row)
        gd_ps = ps.tile([P, B], f32, tag="mm")
        nc.tensor.matmul(gd_ps, ones_row, gd_row, start=True, stop=True)
        GD = singles.tile([P, B], f32, tag="GD")
        nc.vector.tensor_copy(GD, gd_ps)

        # wg = e*gd + pwo   (in place into e)
        nc.vector.tensor_mul(v3(e), v3(e), bcast_inner(GD, R))
        nc.vector.tensor_add(out=e, in0=e, in1=pw)

        # circular 3-tap shift: ws[n] = s1*wg[n] + s0*wg[n-1] + s2*wg[n+1]
        # ta reuses pw's slot (pw died at the wg add); tb reuses ssq's slot
        # (ssq died at the reciprocal)
        ta = big.tile([P, B * R], f32, tag="pw")
        tb = big.tile([P, B * R], f32, tag="ssq")
        wg3, ws3, ta3, tb3 = v3(e), v3(ws), v3(ta), v3(tb)
        nc.vector.tensor_mul(ta3, wg3, bcast_inner(S0, R))
        nc.vector.tensor_mul(tb3, wg3, bcast_inner(S2, R))
        nc.vector.tensor_mul(ws3, wg3, bcast_inner(S1, R))
        # partition carries via circular-shift matmuls on the TensorEngine
        # (issued as soon as ta/tb are ready, overlapping the shifted adds):
        # dn[q, b] = ta[(q-1) mod P, b, R-1];  up[q, b] = tb[(q+1) mod P, b, 0]
        ta_col = AP(ta.tensor, ta.offset + (R - 1), [ta.ap[0], [R, B]])
        tb_col = AP(tb.tensor, tb.offset, [tb.ap[0], [R, B]])
        dn_ps = ps.tile([P, B], f32, tag="mm")
        nc.tensor.matmul(dn_ps, SD, ta_col, start=True, stop=True)
        up_ps = ps.tile([P, B], f32, tag="mm")
        nc.tensor.matmul(up_ps, SU, tb_col, start=True, stop=True)
        nc.vector.tensor_add(
            out=ws3[:, :, 1:R], in0=ws3[:, :, 1:R], in1=ta3[:, :, 0 : R - 1]
        )
        nc.vector.tensor_add(
            out=ws3[:, :, 0 : R - 1], in0=ws3[:, :, 0 : R - 1], in1=tb3[:, :, 1:R]
        )
        nc.vector.tensor_add(
            out=ws3[:, :, 0:1], in0=ws3[:, :, 0:1], in1=bcast_inner(dn_ps, 1)
        )
        nc.vector.tensor_add(
            out=ws3[:, :, R - 1 : R], in0=ws3[:, :, R - 1 : R],
            in1=bcast_inner(up_ps, 1),
        )

        # w_pow = ws ** gamma = exp(gamma * ln(ws))
        nc.scalar.activation(out=ws, in_=ws, func=Act.Ln)
        # preload the Exp table while DVE runs the gamma multiply
        nc.scalar.activation(out=dummy, in_=ks_row[:, 0:1], func=Act.Exp)
        nc.vector.tensor_mul(ws3, ws3, bcast_inner(GAM, R))
        nc.scalar.activation(out=ws, in_=ws, func=Act.Exp)

        # normalize: out = w_pow / (sum + 1e-16)
        cs2 = singles.tile([P, B], f32, tag="cs2")
        nc.vector.tensor_reduce(out=cs2, in_=ws3, axis=Ax.X, op=Alu.add)
        d2_ps = ps.tile([1, B], f32, tag="mm")
        nc.tensor.matmul(d2_ps, ones_col, cs2, start=True, stop=True)
        d2_row = singles.tile([1, B], f32, tag="d2_row")
        nc.vector.tensor_scalar_add(out=d2_row, in0=d2_ps, scalar1=1e-16)
        rd2_row = singles.tile([1, B], f32, tag="rd2_row")
        nc.vector.reciprocal(out=rd2_row, in_=d2_row)
        rd2_ps = ps.tile([P, B], f32, tag="mm")
        nc.tensor.matmul(rd2_ps, ones_row, rd2_row, start=True, stop=True)
        RD2 = singles.tile([P, B], f32, tag="RD2")
        nc.vector.tensor_copy(RD2, rd2_ps)
        nc.vector.tensor_mul(ws3, ws3, bcast_inner(RD2, R))

        nc.sync.dma_start(
            out=out_ap.rearrange("b (p r) -> p b r", r=R),
            in_=ws.rearrange("p (b r) -> p b r", r=R),
        )


def _get_nc():
    if "nc" in _NC_CACHE:
        return _NC_CACHE["nc"]
    from concourse import bacc, mybir

    f32 = mybir.dt.float32
    nc = bacc.Bacc("TRN2", debug=False, num_devices=NCORES)
    ins = {
        "memory": nc.dram_tensor("memory", [B, N, M], f32, kind="ExternalInput").ap(),
        "k": nc.dram_tensor("k", [B, M], f32, kind="ExternalInput").ap(),
        "beta": nc.dram_tensor("beta", [B, 1], f32, kind="ExternalInput").ap(),
        "prev_w": nc.dram_tensor("prev_w", [B, N], f32, kind="ExternalInput").ap(),
        "g": nc.dram_tensor("g", [B, 1], f32, kind="ExternalInput").ap(),
        "s": nc.dram_tensor("s", [B, 3], f32, kind="ExternalInput").ap(),
        "gamma": nc.dram_tensor("gamma", [B, 1], f32, kind="ExternalInput").ap(),
    }
    out_ap = nc.dram_tensor("out", [B, N], f32, kind="ExternalOutput").ap()
    _build_body(nc, out_ap, ins)
    nc.finalize()
    _NC_CACHE["nc"] = nc
    return nc


def _shard_inputs(inputs):
    arrs = {
        name: np.ascontiguousarray(np.asarray(inputs[name], dtype=np.float32))
        for name in ("memory", "k", "beta", "prev_w", "g", "s", "gamma")
    }
    in_maps = []
    for c in range(NCORES):
        sl = slice(c * B, (c + 1) * B)
        in_maps.append({name: np.ascontiguousarray(a[sl]) for name, a in arrs.items()})
    return in_maps


def run(inputs, trace=False):
    from concourse.bass_utils import run_bass_kernel_spmd

    nc = _get_nc()
    in_maps = _shard_inputs(inputs)
    res = run_bass_kernel_spmd(
        nc, in_maps, core_ids=list(range(NCORES)), trace=trace,
        **({"trace_cores": [0]} if trace else {}),
    )
    out = np.concatenate([r["out"] for r in res.results], axis=0)
    return out, res


def kernel(**inputs):
    out, _ = run(inputs, trace=False)
    return out


# revision 44
# speedup vs baseline: 1.4228x; 1.3749x over previous
"""NTM addressing head (nn_HeadBase) Trainium2 Bass kernel.

Full-input contract: kernel(**inputs) takes the unsharded [256, ...] arrays,
shards batch-dim across 8 NeuronCores (pure data parallel), runs one SPMD Bass
program per core, and gathers the full [256, 4096] output.

Per-core layout (B=32 batches, N=4096, M=64):
  memory[b] is streamed as [128, CB*2048] SBUF tiles (CB=4 batches/chunk)
  with n = p*32 + r (partition p, free = (b, r, m)); 8 KB contiguous per
  partition per batch.

  Phase A per chunk: the mem*k multiply is split GpSimd (2.5 batches) / DVE
  (1.5 batches), both writing an fp16 product tile; ACT squares mem into an
  fp16 tile.  The m=64 reductions run as fp16 tensor-tensor halving trees
  (64->32->16->8, 2x DVE rate) finished by a fp32-out native reduce (8->1).
  Emission is software-pipelined: chunk c's mults are emitted before chunk
  c-1's trees so the in-order DVE queue reaches the mult (and frees the mem
  buffer for DMA c+2) without waiting behind tree work.
  Raw k is broadcast to all partitions immediately after its DMA; the
  beta/||k|| scale is applied as one extra phase-B multiply instead of
  delaying the broadcast.

  Phase B (all batches fused as [128, 1024] f32 tiles): a = beta/||k|| *
  dot / sqrt(ssq) (Newton-reciprocal on DVE; exact InstReciprocal costs
  6.5us), softmax (no max-subtract: |a|<1), gated interpolation, 3-tap
  circular shift via shifted APs; the +-1 partition carries go through two
  128x128 circular-shift matmuls on the idle TensorEngine.  pow via exp/ln
  with activation-table preloads hidden behind DVE work.  Per-batch scalars
  are broadcast to [128, B] via K=1 ones-matmuls; PSUM evacuation on DVE.
"""

import numpy as np

B_FULL, N, M = 256, 4096, 64
NCORES = 8
B = B_FULL // NCORES   # 32 batches per core
P = 128                # SBUF partitions
R = N // P             # 32 rows per partition; n = p*R + r

_NC_CACHE = {}


def _build_body(nc, out_ap, ins):
    """Emit the kernel IR. ins: dict name->AP of DRAM inputs, out_ap: DRAM out."""
    from contextlib import ExitStack

    import concourse.bass as bass
    import concourse.tile as tile
    from concourse import mybir

    f32 = mybir.dt.float32
    f16 = mybir.dt.float16
    Alu = mybir.AluOpType
    Act = mybir.ActivationFunctionType
    Ax = mybir.AxisListType
    AP = bass.AP

    mem_ap = ins["memory"]   # [B, N, M]
    k_ap = ins["k"]          # [B, M]
    beta_ap = ins["beta"]    # [B, 1]
    pw_ap = ins["prev_w"]    # [B, N]
    g_ap = ins["g"]          # [B, 1]
    s_ap = ins["s"]          # [B, 3]
    gam_ap = ins["gamma"]    # [B, 1]

    def bcast_inner(ap2d, n):
        # [P, C] -> [P, C, n] with 0-stride inner dim
        return AP(ap2d.tensor, ap2d.offset, list(ap2d.ap) + [[0, n]])

    def row1(ap1d):
        # prepend a unit partition dim to a 1-d AP
        return AP(ap1d.tensor, ap1d.offset, [[0, 1]] + list(ap1d.ap))

    with tile.TileContext(nc) as tc, ExitStack() as ctx:
        singles = ctx.enter_context(tc.tile_pool(name="singles", bufs=1))
        mem_pool = ctx.enter_context(tc.tile_pool(name="mem", bufs=2))
        mth_pool = ctx.enter_context(tc.tile_pool(name="mth", bufs=2))
        pr_pool = ctx.enter_context(tc.tile_pool(name="pr", bufs=2))
        sq_pool = ctx.enter_context(tc.tile_pool(name="sq", bufs=2))
        h_pool = ctx.enter_context(tc.tile_pool(name="h", bufs=1))
        big = ctx.enter_context(tc.tile_pool(name="big", bufs=1))
        ps = ctx.enter_context(tc.tile_pool(name="ps", bufs=2, space="PSUM"))
        ps_big = ctx.enter_context(tc.tile_pool(name="psbig", bufs=1, space="PSUM"))

        # ---- setup ----
        # Small-input DMAs issued from idle engine sequencers (scalar/gpsimd)
        # so the Sync engine's in-order queue is free to start generating the
        # big memory-chunk descriptors immediately.
        ones_col = singles.tile([P, 1], f32, tag="ones_col")
        nc.vector.memset(ones_col, 1.0)
        ones_row = singles.tile([1, P], f32, tag="ones_row")
        nc.vector.memset(ones_row, 1.0)

        # k borrows a rotating memory-chunk slot (it is dead before chunk 2
        # rotates onto this slot; Tile's WAR tracking enforces the ordering)
        k_host = mem_pool.tile([P, 4 * R * M], f32, tag="mt")
        k_row = k_host[0:1, 0 : B * M]
        nc.scalar.dma_start(out=k_row, in_=row1(k_ap.rearrange("b m -> (b m)")))
        b_row = singles.tile([1, B], f32, tag="b_row")
        nc.gpsimd.dma_start(out=b_row, in_=row1(beta_ap.rearrange("b one -> (b one)")))
        g_row = singles.tile([1, B], f32, tag="g_row")
        nc.gpsimd.dma_start(out=g_row, in_=row1(g_ap.rearrange("b one -> (b one)")))
        gm_row = singles.tile([1, B], f32, tag="gm_row")
        nc.gpsimd.dma_start(out=gm_row, in_=row1(gam_ap.rearrange("b one -> (b one)")))
        s_row = singles.tile([1, 3 * B], f32, tag="s_row")
        nc.scalar.dma_start(out=s_row, in_=row1(s_ap.rearrange("b i -> (b i)")))
        # prev_w big tile [P, B*R] in one permuted-AP DMA (128B inner runs)
        pw = big.tile([P, B * R], f32, tag="pw")
        nc.scalar.dma_start(
            out=pw.rearrange("p (b r) -> p b r", r=R),
            in_=pw_ap.rearrange("b (p r) -> p b r", r=R),
        )
        # s_i as [1, B] strided views (stride 3)
        s_perm = s_row.rearrange("p (b i) -> p i b", i=3)
        s_v = [s_perm[:, i, :] for i in range(3)]

        # RAW k broadcast to all partitions ASAP: kb[p, b*M+m] = k[b, m].
        # kb is fp16 so the phase-A multiply runs in the DVE's 2x mode.
        kb_psum = ps_big.tile([P, B * M], f32, tag="kb_psum")
        for j in range(0, B * M, 512):
            nc.tensor.matmul(
                kb_psum[:, j : j + 512], ones_row, k_row[:, j : j + 512],
                start=True, stop=True,
            )
        kb = singles.tile([P, B * M], f16, tag="kb")
        nc.scalar.copy(out=kb, in_=kb_psum)

        # ---- phase A: stream memory in CB-batch chunks ----
        # NOTE: all remaining phase-B setup (bk chain, scalar broadcasts, pwo,
        # shift matrices) is emitted AFTER the last chunk's stream ops so it
        # cannot block the in-order engine queues during phase A.
        CB = 4          # batches per chunk
        NCH = B // CB   # 8 chunks
        dot = big.tile([P, B * R], f32, tag="dot")
        ssq = big.tile([P, B * R], f32, tag="ssq")

        # GpSimd is deliberately UNUSED in phase A: any Q7 activity stalls
        # DVE double-pumped (2x) ops completely (observed on HW), and the
        # whole phase-A pipeline below runs the DVE in 2x mode.  ACT instead
        # converts the stream to fp16 (enabling the 2x multiply) and squares.
        def emit_stream(c):
            """DMA + f16 convert + mult + square for chunk c -> (pr, sq)."""
            b0 = c * CB
            mt = mem_pool.tile([P, CB * R * M], f32, tag="mt")
            mtb = mt.rearrange("p (b f) -> p b f", b=CB)
            # two half-chunk DMAs for finer arrival granularity
            nc.sync.dma_start(
                out=mtb[:, 0:2],
                in_=mem_ap[b0 : b0 + 2].rearrange("b (p r) m -> p b (r m)", p=P),
            )
            nc.sync.dma_start(
                out=mtb[:, 2:4],
                in_=mem_ap[b0 + 2 : b0 + 4].rearrange(
                    "b (p r) m -> p b (r m)", p=P),
            )
            # ACT: fp16 copy of the stream (feeds the 2x multiply), then
            # squares straight from the f32 stream (fp16 out)
            mth = mth_pool.tile([P, CB * R * M], f16, tag="mth")
            mthb = mth.rearrange("p (b f) -> p b f", b=CB)
            nc.scalar.copy(out=mthb[:, 0:2], in_=mtb[:, 0:2])
            nc.scalar.copy(out=mthb[:, 2:4], in_=mtb[:, 2:4])
            sq = sq_pool.tile([P, CB * R * M], f16, tag="sq")
            sqb = sq.rearrange("p (b f) -> p b f", b=CB)
            nc.scalar.square(out=sqb[:, 0:2], in_=mtb[:, 0:2])
            nc.scalar.square(out=sqb[:, 2:4], in_=mtb[:, 2:4])
            # DVE: f16 multiply at 2x
            pr = pr_pool.tile([P, CB * R * M], f16, tag="pr")
            pr4 = pr.rearrange("p (b r m) -> p b r m", b=CB, m=M)
            mth4 = mth.rearrange("p (b r m) -> p b r m", b=CB, m=M)
            kbc = kb[:, b0 * M : (b0 + CB) * M]  # [P, CB*M]
            kb4 = AP(kbc.tensor, kbc.offset, [kbc.ap[0], [M, CB], [0, R], [1, M]])
            nc.vector.tensor_tensor(
                out=pr4, in0=mth4, in1=kb4, op=Alu.mult
            )
            return pr, sq

        def emit_trees(c, pr, sq):
            """fp16 halving trees (2x DVE) + f32-out final reduce for chunk c."""
            b0 = c * CB
            G = CB * R  # 128 groups of 64
            for src, dst in ((pr, dot), (sq, ssq)):
                v64 = src.rearrange("p (g m) -> p g m", m=64)
                h1 = h_pool.tile([P, G * 32], f16, tag="h1")
                h1v = h1.rearrange("p (g m) -> p g m", m=32)
                nc.vector.tensor_add(
                    out=h1v, in0=v64[:, :, 0:32], in1=v64[:, :, 32:64]
                )
                h2 = h_pool.tile([P, G * 16], f16, tag="h2")
                h2v = h2.rearrange("p (g m) -> p g m", m=16)
                nc.vector.tensor_add(
                    out=h2v, in0=h1v[:, :, 0:16], in1=h1v[:, :, 16:32]
                )
                h3 = h_pool.tile([P, G * 8], f16, tag="h3")
                h3v = h3.rearrange("p (g m) -> p g m", m=8)
                nc.vector.tensor_add(
                    out=h3v, in0=h2v[:, :, 0:8], in1=h2v[:, :, 8:16]
                )
                nc.vector.tensor_reduce(
                    out=dst[:, b0 * R : (b0 + CB) * R].rearrange(
                        "p (b r) -> p b r", b=CB),
                    in_=h3v.rearrange("p (b r) m -> p (b r) m", b=CB),
                    axis=Ax.X, op=Alu.add,
                )

        st0 = emit_stream(0)

        # ---- phase-B setup, emitted while the engines wait for chunk 0's
        # DMA (~7us of DVE idle): fills the pipeline-fill window for free ----
        # bk = beta / ||k||; k is squared in place (k_row is dead once the
        # kb broadcast matmuls have read it -- Tile WAR tracking orders this)
        nc.vector.tensor_mul(k_row, k_row, k_row)
        ks_row = singles.tile([1, B], f32, tag="ks_row")
        nc.vector.tensor_reduce(
            out=ks_row, in_=k_row.rearrange("p (b m) -> p b m", m=M),
            axis=Ax.X, op=Alu.add,
        )
        # this Sqrt also preloads the table for the big ssq sqrt below
        kn_row = singles.tile([1, B], f32, tag="kn_row")
        nc.scalar.activation(out=kn_row, in_=ks_row, func=Act.Sqrt)
        rk_row = singles.tile([1, B], f32, tag="rk_row")
        nc.vector.reciprocal(out=rk_row, in_=kn_row)
        bk_row = singles.tile([1, B], f32, tag="bk_row")
        nc.vector.tensor_mul(bk_row, b_row, rk_row)
        # omg = 1 - g
        omg_row = singles.tile([1, B], f32, tag="omg_row")
        nc.vector.tensor_scalar(
            out=omg_row, in0=g_row, scalar1=-1.0, scalar2=1.0,
            op0=Alu.mult, op1=Alu.add,
        )
        # broadcast round 1: [bk, omg, s0, s1, s2, gamma] -> [P, 6*B]
        NSC = 6
        asm1 = singles.tile([1, NSC * B], f32, tag="asm1")
        for i, src in enumerate([bk_row, omg_row, s_v[0], s_v[1], s_v[2], gm_row]):
            nc.vector.tensor_copy(asm1[:, i * B : (i + 1) * B], src)
        bc1_ps = ps.tile([P, NSC * B], f32, tag="mm")
        nc.tensor.matmul(bc1_ps, ones_row, asm1, start=True, stop=True)
        BC1 = singles.tile([P, NSC * B], f32, tag="BC1")
        nc.vector.tensor_copy(BC1, bc1_ps)
        BK = BC1[:, 0 * B : 1 * B]
        OMG = BC1[:, 1 * B : 2 * B]
        S0 = BC1[:, 2 * B : 3 * B]
        S1 = BC1[:, 3 * B : 4 * B]
        S2 = BC1[:, 4 * B : 5 * B]
        GAM = BC1[:, 5 * B : 6 * B]
        # pwo = prev_w * (1 - g)
        nc.vector.tensor_mul(
            pw.rearrange("p (b r) -> p b r", r=R),
            pw.rearrange("p (b r) -> p b r", r=R),
            bcast_inner(OMG, R),
        )

        # circular +-1 partition shift matrices for the conv carries (on the
        # DVE, still inside the chunk-0 fill window; GpSimd must stay idle):
        # SD[p, q] = 1 iff q == (p+1) mod P ; SU[p, q] = 1 iff q == (p-1) mod P
        ones_sq = singles.tile([P, P], f32, tag="ones_sq")
        nc.vector.memset(ones_sq, 1.0)
        sd_t = singles.tile([P, P], f32, tag="sd_t")
        nc.gpsimd.affine_select(
            out=sd_t, in_=ones_sq, pattern=[[1, P]], compare_op=Alu.is_equal,
            fill=0.0, base=-1, channel_multiplier=-1,
        )
        SD = singles.tile([P, P], f32, tag="SD")
        nc.gpsimd.affine_select(
            out=SD, in_=sd_t, pattern=[[1, P]], compare_op=Alu.not_equal,
            fill=1.0, base=P - 1, channel_multiplier=-1,
        )
        su_t = singles.tile([P, P], f32, tag="sd_t")
        nc.gpsimd.affine_select(
            out=su_t, in_=ones_sq, pattern=[[1, P]], compare_op=Alu.is_equal,
            fill=0.0, base=1, channel_multiplier=-1,
        )
        SU = singles.tile([P, P], f32, tag="SU")
        nc.gpsimd.affine_select(
            out=SU, in_=su_t, pattern=[[1, P]], compare_op=Alu.not_equal,
            fill=1.0, base=-(P - 1), channel_multiplier=-1,
        )

        emit_trees(0, *st0)
        for c in range(1, NCH):
            st = emit_stream(c)
            emit_trees(c, *st)

        # re-preload the Sqrt table after the last square so the phase-B sqrt
        # doesn't pay the switch (the kn sqrt above loaded it too early)
        dummy = singles.tile([1, 1], f32, tag="dummy")
        nc.scalar.activation(out=dummy, in_=ks_row[:, 0:1], func=Act.Sqrt)

        # ---- phase B ----
        def v3(t):
            return t.rearrange("p (b r) -> p b r", r=R)

        # a = (beta/||k||) * dot / sqrt(ssq)
        # rstd lands in ws (overwritten later by the shift); the Newton
        # scratch bitcasts a dead fp16 product tile.
        nc.scalar.activation(out=ssq, in_=ssq, func=Act.Sqrt)
        # preload the Exp table while DVE runs the reciprocal
        nc.scalar.activation(out=dummy, in_=ks_row[:, 0:1], func=Act.Exp)
        ws = big.tile([P, B * R], f32, tag="ws")
        scr16 = sq_pool.tile([P, CB * R * M], f16, tag="sq")
        scr = scr16[:, 0 : 2 * B * R].bitcast(f32)
        nc.vector.reciprocal_approx_accurate(out=ws, in_=ssq, scratch=scr)
        nc.vector.tensor_mul(dot, dot, ws)
        nc.vector.tensor_mul(v3(dot), v3(dot), bcast_inner(BK, R))

        # e = exp(a), in place
        nc.scalar.activation(out=dot, in_=dot, func=Act.Exp)
        e = dot
        # preload the Ln table while DVE runs the softmax/gating chain
        nc.scalar.activation(out=dummy, in_=ks_row[:, 0:1], func=Act.Ln)

        # denom per batch; gd = g/denom
        cs = singles.tile([P, B], f32, tag="cs")
        nc.vector.tensor_reduce(out=cs, in_=v3(e), axis=Ax.X, op=Alu.add)
        den_ps = ps.tile([1, B], f32, tag="mm")
        nc.tensor.matmul(den_ps, ones_col, cs, start=True, stop=True)
        rden_row = singles.tile([1, B], f32, tag="rden_row")
        nc.vector.reciprocal(out=rden_row, in_=den_ps)
        gd_row = singles.tile([1, B], f32, tag="gd_row")
        nc.vector.tensor_mul(gd_row, rden_row, g_row)
        gd_ps = ps.tile([P, B], f32, tag="mm")
        nc.tensor.matmul(gd_ps, ones_row, gd_row, start=True, stop=True)
        GD = singles.tile([P, B], f32, tag="GD")
        nc.vector.tensor_copy(GD, gd_ps)

        # wg = e*gd + pwo   (in place into e)
        nc.vector.tensor_mul(v3(e), v3(e), bcast_inner(GD, R))
        nc.vector.tensor_add(out=e, in0=e, in1=pw)

        # circular 3-tap shift: ws[n] = s1*wg[n] + s0*wg[n-1] + s2*wg[n+1]
        # ta reuses pw's slot (pw died at the wg add); tb reuses ssq's slot
        # (ssq died at the reciprocal)
        ta = big.tile([P, B * R], f32, tag="pw")
        tb = big.tile([P, B * R], f32, tag="ssq")
        wg3, ws3, ta3, tb3 = v3(e), v3(ws), v3(ta), v3(tb)
        nc.vector.tensor_mul(ta3, wg3, bcast_inner(S0, R))
        nc.vector.tensor_mul(tb3, wg3, bcast_inner(S2, R))
        nc.vector.tensor_mul(ws3, wg3, bcast_inner(S1, R))
        # partition carries via circular-shift matmuls on the TensorEngine
        # (issued as soon as ta/tb are ready, overlapping the shifted adds):
        # dn[q, b] = ta[(q-1) mod P, b, R-1];  up[q, b] = tb[(q+1) mod P, b, 0]
        ta_col = AP(ta.tensor, ta.offset + (R - 1), [ta.ap[0], [R, B]])
        tb_col = AP(tb.tensor, tb.offset, [tb.ap[0], [R, B]])
        dn_ps = ps.tile([P, B], f32, tag="mm")
        nc.tensor.matmul(dn_ps, SD, ta_col, start=True, stop=True)
        up_ps = ps.tile([P, B], f32, tag="mm")
        nc.tensor.matmul(up_ps, SU, tb_col, start=True, stop=True)
        nc.vector.tensor_add(
            out=ws3[:, :, 1:R], in0=ws3[:, :, 1:R], in1=ta3[:, :, 0 : R - 1]
        )
        nc.vector.tensor_add(
            out=ws3[:, :, 0 : R - 1], in0=ws3[:, :, 0 : R - 1], in1=tb3[:, :, 1:R]
        )
        nc.vector.tensor_add(
            out=ws3[:, :, 0:1], in0=ws3[:, :, 0:1], in1=bcast_inner(dn_ps, 1)
        )
        nc.vector.tensor_add(
            out=ws3[:, :, R - 1 : R], in0=ws3[:, :, R - 1 : R],
            in1=bcast_inner(up_ps, 1),
        )

        # w_pow = ws ** gamma = exp(gamma * ln(ws))
        nc.scalar.activation(out=ws, in_=ws, func=Act.Ln)
        # preload the Exp table while DVE runs the gamma multiply
        nc.scalar.activation(out=dummy, in_=ks_row[:, 0:1], func=Act.Exp)
        nc.vector.tensor_mul(ws3, ws3, bcast_inner(GAM, R))
        nc.scalar.activation(out=ws, in_=ws, func=Act.Exp)

        # normalize: out = w_pow / (sum + 1e-16)
        cs2 = singles.tile([P, B], f32, tag="cs2")
        nc.vector.tensor_reduce(out=cs2, in_=ws3, axis=Ax.X, op=Alu.add)
        d2_ps = ps.tile([1, B], f32, tag="mm")
        nc.tensor.matmul(d2_ps, ones_col, cs2, start=True, stop=True)
        d2_row = singles.tile([1, B], f32, tag="d2_row")
        nc.vector.tensor_scalar_add(out=d2_row, in0=d2_ps, scalar1=1e-16)
        rd2_row = singles.tile([1, B], f32, tag="rd2_row")
        nc.vector.reciprocal(out=rd2_row, in_=d2_row)
        rd2_ps = ps.tile([P, B], f32, tag="mm")
        nc.tensor.matmul(rd2_ps, ones_row, rd2_row, start=True, stop=True)
        RD2 = singles.tile([P, B], f32, tag="RD2")
        nc.vector.tensor_copy(RD2, rd2_ps)
        nc.vector.tensor_mul(ws3, ws3, bcast_inner(RD2, R))

        nc.sync.dma_start(
            out=out_ap.rearrange("b (p r) -> p b r", r=R),
            in_=ws.rearrange("p (b r) -> p b r", r=R),
        )


def _get_nc():
    if "nc" in _NC_CACHE:
        return _NC_CACHE["nc"]
    from concourse import bacc, mybir

    f32 = mybir.dt.float32
    nc = bacc.Bacc("TRN2", debug=False, num_devices=NCORES)
    ins = {
        "memory": nc.dram_tensor("memory", [B, N, M], f32, kind="ExternalInput").ap(),
        "k": nc.dram_tensor("k", [B, M], f32, kind="ExternalInput").ap(),
        "beta": nc.dram_tensor("beta", [B, 1], f32, kind="ExternalInput").ap(),
        "prev_w": nc.dram_tensor("prev_w", [B, N], f32, kind="ExternalInput").ap(),
        "g": nc.dram_tensor("g", [B, 1], f32, kind="ExternalInput").ap(),
        "s": nc.dram_tensor("s", [B, 3], f32, kind="ExternalInput").ap(),
        "gamma": nc.dram_tensor("gamma", [B, 1], f32, kind="ExternalInput").ap(),
    }
    out_ap = nc.dram_tensor("out", [B, N], f32, kind="ExternalOutput").ap()
    _build_body(nc, out_ap, ins)
    nc.finalize()
    _NC_CACHE["nc"] = nc
    return nc


def _shard_inputs(inputs):
    arrs = {
        name: np.ascontiguousarray(np.asarray(inputs[name], dtype=np.float32))
        for name in ("memory", "k", "beta", "prev_w", "g", "s", "gamma")
    }
    in_maps = []
    for c in range(NCORES):
        sl = slice(c * B, (c + 1) * B)
        in_maps.append({name: np.ascontiguousarray(a[sl]) for name, a in arrs.items()})
    return in_maps


def run(inputs, trace=False):
    from concourse.bass_utils import run_bass_kernel_spmd

    nc = _get_nc()
    in_maps = _shard_inputs(inputs)
    res = run_bass_kernel_spmd(
        nc, in_maps, core_ids=list(range(NCORES)), trace=trace,
        **({"trace_cores": [0]} if trace else {}),
    )
    out = np.concatenate([r["out"] for r in res.results], axis=0)
    return out, res


def kernel(**inputs):
    out, _ = run(inputs, trace=False)
    return out


# revision 48
# speedup vs baseline: 1.4311x; 1.0058x over previous
"""NTM addressing head (nn_HeadBase) Trainium2 Bass kernel.

Full-input contract: kernel(**inputs) takes the unsharded [256, ...] arrays,
shards batch-dim across 8 NeuronCores (pure data parallel), runs one SPMD Bass
program per core, and gathers the full [256, 4096] output.

Per-core layout (B=32 batches, N=4096, M=64):
  memory[b] is streamed as [128, CB*2048] SBUF tiles (CB=4 batches/chunk)
  with n = p*32 + r (partition p, free = (b, r, m)); 8 KB contiguous per
  partition per batch.

  Phase A per chunk: the mem*k multiply is split GpSimd (2.5 batches) / DVE
  (1.5 batches), both writing an fp16 product tile; ACT squares mem into an
  fp16 tile.  The m=64 reductions run as fp16 tensor-tensor halving trees
  (64->32->16->8, 2x DVE rate) finished by a fp32-out native reduce (8->1).
  Emission is software-pipelined: chunk c's mults are emitted before chunk
  c-1's trees so the in-order DVE queue reaches the mult (and frees the mem
  buffer for DMA c+2) without waiting behind tree work.
  Raw k is broadcast to all partitions immediately after its DMA; the
  beta/||k|| scale is applied as one extra phase-B multiply instead of
  delaying the broadcast.

  Phase B (all batches fused as [128, 1024] f32 tiles): a = beta/||k|| *
  dot / sqrt(ssq) (Newton-reciprocal on DVE; exact InstReciprocal costs
  6.5us), softmax (no max-subtract: |a|<1), gated interpolation, 3-tap
  circular shift via shifted APs; the +-1 partition carries go through two
  128x128 circular-shift matmuls on the idle TensorEngine.  pow via exp/ln
  with activation-table preloads hidden behind DVE work.  Per-batch scalars
  are broadcast to [128, B] via K=1 ones-matmuls; PSUM evacuation on DVE.
"""

import numpy as np

B_FULL, N, M = 256, 4096, 64
NCORES = 8
B = B_FULL // NCORES   # 32 batches per core
P = 128                # SBUF partitions
R = N // P             # 32 rows per partition; n = p*R + r

_NC_CACHE = {}


def _build_body(nc, out_ap, ins):
    """Emit the kernel IR. ins: dict name->AP of DRAM inputs, out_ap: DRAM out."""
    from contextlib import ExitStack

    import concourse.bass as bass
    import concourse.tile as tile
    from concourse import mybir

    f32 = mybir.dt.float32
    f16 = mybir.dt.float16
    Alu = mybir.AluOpType
    Act = mybir.ActivationFunctionType
    Ax = mybir.AxisListType
    AP = bass.AP

    mem_ap = ins["memory"]   # [B, N, M]
    k_ap = ins["k"]          # [B, M]
    beta_ap = ins["beta"]    # [B, 1]
    pw_ap = ins["prev_w"]    # [B, N]
    g_ap = ins["g"]          # [B, 1]
    s_ap = ins["s"]          # [B, 3]
    gam_ap = ins["gamma"]    # [B, 1]

    def bcast_inner(ap2d, n):
        # [P, C] -> [P, C, n] with 0-stride inner dim
        return AP(ap2d.tensor, ap2d.offset, list(ap2d.ap) + [[0, n]])

    def row1(ap1d):
        # prepend a unit partition dim to a 1-d AP
        return AP(ap1d.tensor, ap1d.offset, [[0, 1]] + list(ap1d.ap))

    with tile.TileContext(nc) as tc, ExitStack() as ctx:
        singles = ctx.enter_context(tc.tile_pool(name="singles", bufs=1))
        mem_pool = ctx.enter_context(tc.tile_pool(name="mem", bufs=2))
        mth_pool = ctx.enter_context(tc.tile_pool(name="mth", bufs=2))
        pr_pool = ctx.enter_context(tc.tile_pool(name="pr", bufs=2))
        sq_pool = ctx.enter_context(tc.tile_pool(name="sq", bufs=2))
        h_pool = ctx.enter_context(tc.tile_pool(name="h", bufs=1))
        big = ctx.enter_context(tc.tile_pool(name="big", bufs=1))
        ps = ctx.enter_context(tc.tile_pool(name="ps", bufs=2, space="PSUM"))
        ps_big = ctx.enter_context(tc.tile_pool(name="psbig", bufs=1, space="PSUM"))

        # ---- setup ----
        # Small-input DMAs issued from idle engine sequencers (scalar/gpsimd)
        # so the Sync engine's in-order queue is free to start generating the
        # big memory-chunk descriptors immediately.
        ones_col = singles.tile([P, 1], f32, tag="ones_col")
        nc.vector.memset(ones_col, 1.0)
        ones_row = singles.tile([1, P], f32, tag="ones_row")
        nc.vector.memset(ones_row, 1.0)

        # All small-input DMAs on GpSimd's sequencer (idle, and GP activity
        # at t<15us predates any DVE 2x op): ACT's queue stays free for the
        # kb copy + chunk-0 converts, Sync's for chunk descriptors.
        # k borrows a rotating memory-chunk slot (it is dead before chunk 2
        # rotates onto this slot; Tile's WAR tracking enforces the ordering)
        k_host = mem_pool.tile([P, 4 * R * M], f32, tag="mt")
        k_row = k_host[0:1, 0 : B * M]
        nc.gpsimd.dma_start(out=k_row, in_=row1(k_ap.rearrange("b m -> (b m)")))
        b_row = singles.tile([1, B], f32, tag="b_row")
        nc.gpsimd.dma_start(out=b_row, in_=row1(beta_ap.rearrange("b one -> (b one)")))
        g_row = singles.tile([1, B], f32, tag="g_row")
        nc.gpsimd.dma_start(out=g_row, in_=row1(g_ap.rearrange("b one -> (b one)")))
        gm_row = singles.tile([1, B], f32, tag="gm_row")
        nc.gpsimd.dma_start(out=gm_row, in_=row1(gam_ap.rearrange("b one -> (b one)")))
        s_row = singles.tile([1, 3 * B], f32, tag="s_row")
        nc.gpsimd.dma_start(out=s_row, in_=row1(s_ap.rearrange("b i -> (b i)")))
        # prev_w big tile [P, B*R] in one permuted-AP DMA (128B inner runs)
        pw = big.tile([P, B * R], f32, tag="pw")
        nc.gpsimd.dma_start(
            out=pw.rearrange("p (b r) -> p b r", r=R),
            in_=pw_ap.rearrange("b (p r) -> p b r", r=R),
        )
        # s_i as [1, B] strided views (stride 3)
        s_perm = s_row.rearrange("p (b i) -> p i b", i=3)
        s_v = [s_perm[:, i, :] for i in range(3)]

        # RAW k broadcast to all partitions ASAP: kb[p, b*M+m] = k[b, m].
        # kb is fp16 so the phase-A multiply runs in the DVE's 2x mode.
        kb_psum = ps_big.tile([P, B * M], f32, tag="kb_psum")
        for j in range(0, B * M, 512):
            nc.tensor.matmul(
                kb_psum[:, j : j + 512], ones_row, k_row[:, j : j + 512],
                start=True, stop=True,
            )
        kb = singles.tile([P, B * M], f16, tag="kb")
        nc.scalar.copy(out=kb, in_=kb_psum)

        # ---- phase A: stream memory in CB-batch chunks ----
        # NOTE: all remaining phase-B setup (bk chain, scalar broadcasts, pwo,
        # shift matrices) is emitted AFTER the last chunk's stream ops so it
        # cannot block the in-order engine queues during phase A.
        CB = 4          # batches per chunk
        NCH = B // CB   # 8 chunks
        dot = big.tile([P, B * R], f32, tag="dot")
        ssq = big.tile([P, B * R], f32, tag="ssq")

        # GpSimd is deliberately UNUSED in phase A: any Q7 activity stalls
        # DVE double-pumped (2x) ops completely (observed on HW), and the
        # whole phase-A pipeline below runs the DVE in 2x mode.  ACT instead
        # converts the stream to fp16 (enabling the 2x multiply) and squares.
        def emit_stream(c):
            """DMA + f16 convert + mult + square for chunk c -> (pr, sq)."""
            b0 = c * CB
            mt = mem_pool.tile([P, CB * R * M], f32, tag="mt")
            mtb = mt.rearrange("p (b f) -> p b f", b=CB)
            # two half-chunk DMAs for finer arrival granularity
            nc.sync.dma_start(
                out=mtb[:, 0:2],
                in_=mem_ap[b0 : b0 + 2].rearrange("b (p r) m -> p b (r m)", p=P),
            )
            nc.sync.dma_start(
                out=mtb[:, 2:4],
                in_=mem_ap[b0 + 2 : b0 + 4].rearrange(
                    "b (p r) m -> p b (r m)", p=P),
            )
            # ACT: fp16 copy of the stream (feeds the 2x multiply), then
            # squares straight from the f32 stream (fp16 out)
            mth = mth_pool.tile([P, CB * R * M], f16, tag="mth")
            mthb = mth.rearrange("p (b f) -> p b f", b=CB)
            nc.scalar.copy(out=mthb[:, 0:2], in_=mtb[:, 0:2])
            nc.scalar.copy(out=mthb[:, 2:4], in_=mtb[:, 2:4])
            sq = sq_pool.tile([P, CB * R * M], f16, tag="sq")
            sqb = sq.rearrange("p (b f) -> p b f", b=CB)
            nc.scalar.square(out=sqb[:, 0:2], in_=mtb[:, 0:2])
            nc.scalar.square(out=sqb[:, 2:4], in_=mtb[:, 2:4])
            # DVE: f16 multiply at 2x
            pr = pr_pool.tile([P, CB * R * M], f16, tag="pr")
            pr4 = pr.rearrange("p (b r m) -> p b r m", b=CB, m=M)
            mth4 = mth.rearrange("p (b r m) -> p b r m", b=CB, m=M)
            kbc = kb[:, b0 * M : (b0 + CB) * M]  # [P, CB*M]
            kb4 = AP(kbc.tensor, kbc.offset, [kbc.ap[0], [M, CB], [0, R], [1, M]])
            nc.vector.tensor_tensor(
                out=pr4, in0=mth4, in1=kb4, op=Alu.mult
            )
            return pr, sq

        def emit_trees(c, pr, sq):
            """fp16 halving trees (2x DVE) + f32-out final reduce for chunk c."""
            b0 = c * CB
            G = CB * R  # 128 groups of 64
            for src, dst in ((pr, dot), (sq, ssq)):
                v64 = src.rearrange("p (g m) -> p g m", m=64)
                h1 = h_pool.tile([P, G * 32], f16, tag="h1")
                h1v = h1.rearrange("p (g m) -> p g m", m=32)
                nc.vector.tensor_add(
                    out=h1v, in0=v64[:, :, 0:32], in1=v64[:, :, 32:64]
                )
                h2 = h_pool.tile([P, G * 16], f16, tag="h2")
                h2v = h2.rearrange("p (g m) -> p g m", m=16)
                nc.vector.tensor_add(
                    out=h2v, in0=h1v[:, :, 0:16], in1=h1v[:, :, 16:32]
                )
                h3 = h_pool.tile([P, G * 8], f16, tag="h3")
                h3v = h3.rearrange("p (g m) -> p g m", m=8)
                nc.vector.tensor_add(
                    out=h3v, in0=h2v[:, :, 0:8], in1=h2v[:, :, 8:16]
                )
                nc.vector.tensor_reduce(
                    out=dst[:, b0 * R : (b0 + CB) * R].rearrange(
                        "p (b r) -> p b r", b=CB),
                    in_=h3v.rearrange("p (b r) m -> p (b r) m", b=CB),
                    axis=Ax.X, op=Alu.add,
                )

        st0 = emit_stream(0)

        # ---- phase-B setup, emitted while the engines wait for chunk 0's
        # DMA (~7us of DVE idle): fills the pipeline-fill window for free ----
        # bk = beta / ||k||; k is squared in place (k_row is dead once the
        # kb broadcast matmuls have read it -- Tile WAR tracking orders this)
        nc.vector.tensor_mul(k_row, k_row, k_row)
        ks_row = singles.tile([1, B], f32, tag="ks_row")
        nc.vector.tensor_reduce(
            out=ks_row, in_=k_row.rearrange("p (b m) -> p b m", m=M),
            axis=Ax.X, op=Alu.add,
        )
        # this Sqrt also preloads the table for the big ssq sqrt below
        kn_row = singles.tile([1, B], f32, tag="kn_row")
        nc.scalar.activation(out=kn_row, in_=ks_row, func=Act.Sqrt)
        rk_row = singles.tile([1, B], f32, tag="rk_row")
        nc.vector.reciprocal(out=rk_row, in_=kn_row)
        bk_row = singles.tile([1, B], f32, tag="bk_row")
        nc.vector.tensor_mul(bk_row, b_row, rk_row)
        # omg = 1 - g
        omg_row = singles.tile([1, B], f32, tag="omg_row")
        nc.vector.tensor_scalar(
            out=omg_row, in0=g_row, scalar1=-1.0, scalar2=1.0,
            op0=Alu.mult, op1=Alu.add,
        )
        # broadcast round 1: [bk, omg, s0, s1, s2, gamma] -> [P, 6*B]
        NSC = 6
        asm1 = singles.tile([1, NSC * B], f32, tag="asm1")
        for i, src in enumerate([bk_row, omg_row, s_v[0], s_v[1], s_v[2], gm_row]):
            nc.vector.tensor_copy(asm1[:, i * B : (i + 1) * B], src)
        bc1_ps = ps.tile([P, NSC * B], f32, tag="mm")
        nc.tensor.matmul(bc1_ps, ones_row, asm1, start=True, stop=True)
        BC1 = singles.tile([P, NSC * B], f32, tag="BC1")
        nc.vector.tensor_copy(BC1, bc1_ps)
        BK = BC1[:, 0 * B : 1 * B]
        OMG = BC1[:, 1 * B : 2 * B]
        S0 = BC1[:, 2 * B : 3 * B]
        S1 = BC1[:, 3 * B : 4 * B]
        S2 = BC1[:, 4 * B : 5 * B]
        GAM = BC1[:, 5 * B : 6 * B]
        # pwo = prev_w * (1 - g), in fp16 for the 2x gating chain
        pwo = singles.tile([P, B * R], f16, tag="pwo")
        nc.vector.tensor_mul(
            pwo.rearrange("p (b r) -> p b r", r=R),
            pw.rearrange("p (b r) -> p b r", r=R),
            bcast_inner(OMG, R),
        )
        # fp16 copies of the shift taps for the 2x shift chain
        Sh = singles.tile([P, 3 * B], f16, tag="Sh")
        nc.vector.tensor_copy(Sh, BC1[:, 2 * B : 5 * B])
        S0h = Sh[:, 0 * B : 1 * B]
        S1h = Sh[:, 1 * B : 2 * B]
        S2h = Sh[:, 2 * B : 3 * B]

        # circular +-1 partition shift matrices for the conv carries (on the
        # DVE, still inside the chunk-0 fill window; GpSimd must stay idle):
        # SD[p, q] = 1 iff q == (p+1) mod P ; SU[p, q] = 1 iff q == (p-1) mod P
        ones_sq = singles.tile([P, P], f16, tag="ones_sq")
        nc.vector.memset(ones_sq, 1.0)
        sd_t = singles.tile([P, P], f16, tag="sd_t")
        nc.gpsimd.affine_select(
            out=sd_t, in_=ones_sq, pattern=[[1, P]], compare_op=Alu.is_equal,
            fill=0.0, base=-1, channel_multiplier=-1,
        )
        SD = singles.tile([P, P], f16, tag="SD")
        nc.gpsimd.affine_select(
            out=SD, in_=sd_t, pattern=[[1, P]], compare_op=Alu.not_equal,
            fill=1.0, base=P - 1, channel_multiplier=-1,
        )
        su_t = singles.tile([P, P], f16, tag="sd_t")
        nc.gpsimd.affine_select(
            out=su_t, in_=ones_sq, pattern=[[1, P]], compare_op=Alu.is_equal,
            fill=0.0, base=1, channel_multiplier=-1,
        )
        SU = singles.tile([P, P], f16, tag="SU")
        nc.gpsimd.affine_select(
            out=SU, in_=su_t, pattern=[[1, P]], compare_op=Alu.not_equal,
            fill=1.0, base=-(P - 1), channel_multiplier=-1,
        )

        emit_trees(0, *st0)
        for c in range(1, NCH):
            st = emit_stream(c)
            emit_trees(c, *st)

        # re-preload the Sqrt table after the last square so the phase-B sqrt
        # doesn't pay the switch (the kn sqrt above loaded it too early)
        dummy = singles.tile([1, 1], f32, tag="dummy")
        nc.scalar.activation(out=dummy, in_=ks_row[:, 0:1], func=Act.Sqrt)

        # ---- phase B ----
        def v3(t):
            return t.rearrange("p (b r) -> p b r", r=R)

        # a = (beta/||k||) * dot / sqrt(ssq)
        # rstd lands in ws (overwritten later by the shift); the Newton
        # scratch bitcasts a dead fp16 product tile.
        nc.scalar.activation(out=ssq, in_=ssq, func=Act.Sqrt)
        # preload the Exp table while DVE runs the reciprocal
        nc.scalar.activation(out=dummy, in_=ks_row[:, 0:1], func=Act.Exp)
        ws = big.tile([P, B * R], f32, tag="ws")
        scr16 = sq_pool.tile([P, CB * R * M], f16, tag="sq")
        scr = scr16[:, 0 : 2 * B * R].bitcast(f32)
        nc.vector.reciprocal_approx_accurate(out=ws, in_=ssq, scratch=scr)
        nc.vector.tensor_mul(dot, dot, ws)
        nc.vector.tensor_mul(v3(dot), v3(dot), bcast_inner(BK, R))

        # fp16 scratch for the gating/shift chain, carved out of the dead
        # fp16 square tile: the DVE runs these element-wise ops at 2x
        # (GpSimd is idle here, so the 2x/Q7 conflict cannot bite).
        e16 = scr16[:, 0 * B * R : 1 * B * R]
        ws16 = scr16[:, 1 * B * R : 2 * B * R]
        ta16 = scr16[:, 2 * B * R : 3 * B * R]
        tb16 = scr16[:, 3 * B * R : 4 * B * R]
        wp16 = scr16[:, 4 * B * R : 5 * B * R]

        # e = exp(a), fp16
        nc.scalar.activation(out=e16, in_=dot, func=Act.Exp)
        # preload the Ln table while DVE runs the softmax/gating chain
        nc.scalar.activation(out=dummy, in_=ks_row[:, 0:1], func=Act.Ln)

        # denom per batch; gd = g/denom
        cs = singles.tile([P, B], f32, tag="cs")
        nc.vector.tensor_reduce(out=cs, in_=v3(e16), axis=Ax.X, op=Alu.add)
        den_ps = ps.tile([1, B], f32, tag="mm")
        nc.tensor.matmul(den_ps, ones_col, cs, start=True, stop=True)
        rden_row = singles.tile([1, B], f32, tag="rden_row")
        nc.vector.reciprocal(out=rden_row, in_=den_ps)
        gd_row = singles.tile([1, B], f32, tag="gd_row")
        nc.vector.tensor_mul(gd_row, rden_row, g_row)
        gd_ps = ps.tile([P, B], f32, tag="mm")
        nc.tensor.matmul(gd_ps, ones_row, gd_row, start=True, stop=True)
        GDh = singles.tile([P, B], f16, tag="GDh")
        nc.vector.tensor_copy(GDh, gd_ps)

        # wg = e*gd + pwo   (in place into e16)
        nc.vector.tensor_mul(v3(e16), v3(e16), bcast_inner(GDh, R))
        nc.vector.tensor_add(out=e16, in0=e16, in1=pwo)

        # circular 3-tap shift: ws[n] = s1*wg[n] + s0*wg[n-1] + s2*wg[n+1]
        wg3, ws3h, ta3, tb3 = v3(e16), v3(ws16), v3(ta16), v3(tb16)
        nc.vector.tensor_mul(ta3, wg3, bcast_inner(S0h, R))
        nc.vector.tensor_mul(tb3, wg3, bcast_inner(S2h, R))
        nc.vector.tensor_mul(ws3h, wg3, bcast_inner(S1h, R))
        # partition carries via circular-shift matmuls on the TensorEngine
        # (issued as soon as ta/tb are ready, overlapping the shifted adds):
        # dn[q, b] = ta[(q-1) mod P, b, R-1];  up[q, b] = tb[(q+1) mod P, b, 0]
        ta_col = AP(ta16.tensor, ta16.offset + (R - 1), [ta16.ap[0], [R, B]])
        tb_col = AP(tb16.tensor, tb16.offset, [tb16.ap[0], [R, B]])
        dn_ps = ps.tile([P, B], f32, tag="mm")
        nc.tensor.matmul(dn_ps, SD, ta_col, start=True, stop=True)
        up_ps = ps.tile([P, B], f32, tag="mm")
        nc.tensor.matmul(up_ps, SU, tb_col, start=True, stop=True)
        nc.vector.tensor_add(
            out=ws3h[:, :, 1:R], in0=ws3h[:, :, 1:R], in1=ta3[:, :, 0 : R - 1]
        )
        nc.vector.tensor_add(
            out=ws3h[:, :, 0 : R - 1], in0=ws3h[:, :, 0 : R - 1],
            in1=tb3[:, :, 1:R],
        )
        nc.vector.tensor_add(
            out=ws3h[:, :, 0:1], in0=ws3h[:, :, 0:1], in1=bcast_inner(dn_ps, 1)
        )
        nc.vector.tensor_add(
            out=ws3h[:, :, R - 1 : R], in0=ws3h[:, :, R - 1 : R],
            in1=bcast_inner(up_ps, 1),
        )

        # w_pow = ws ** gamma = exp(gamma * ln(ws))
        nc.scalar.activation(out=ws, in_=ws16, func=Act.Ln)
        # preload the Exp table while DVE runs the gamma multiply
        nc.scalar.activation(out=dummy, in_=ks_row[:, 0:1], func=Act.Exp)
        ws3 = v3(ws)
        nc.vector.tensor_mul(ws3, ws3, bcast_inner(GAM, R))
        nc.scalar.activation(out=wp16, in_=ws, func=Act.Exp)

        # normalize: out = w_pow / (sum + 1e-16)
        cs2 = singles.tile([P, B], f32, tag="cs2")
        nc.vector.tensor_reduce(out=cs2, in_=v3(wp16), axis=Ax.X, op=Alu.add)
        d2_ps = ps.tile([1, B], f32, tag="mm")
        nc.tensor.matmul(d2_ps, ones_col, cs2, start=True, stop=True)
        d2_row = singles.tile([1, B], f32, tag="d2_row")
        nc.vector.tensor_scalar_add(out=d2_row, in0=d2_ps, scalar1=1e-16)
        rd2_row = singles.tile([1, B], f32, tag="rd2_row")
        nc.vector.reciprocal(out=rd2_row, in_=d2_row)
        rd2_ps = ps.tile([P, B], f32, tag="mm")
        nc.tensor.matmul(rd2_ps, ones_row, rd2_row, start=True, stop=True)
        RD2 = singles.tile([P, B], f32, tag="RD2")
        nc.vector.tensor_copy(RD2, rd2_ps)
        nc.vector.tensor_mul(ws3, v3(wp16), bcast_inner(RD2, R))

        nc.sync.dma_start(
            out=out_ap.rearrange("b (p r) -> p b r", r=R),
            in_=ws.rearrange("p (b r) -> p b r", r=R),
        )


def _get_nc():
    if "nc" in _NC_CACHE:
        return _NC_CACHE["nc"]
    from concourse import bacc, mybir

    f32 = mybir.dt.float32
    nc = bacc.Bacc("TRN2", debug=False, num_devices=NCORES)
    ins = {
        "memory": nc.dram_tensor("memory", [B, N, M], f32, kind="ExternalInput").ap(),
        "k": nc.dram_tensor("k", [B, M], f32, kind="ExternalInput").ap(),
        "beta": nc.dram_tensor("beta", [B, 1], f32, kind="ExternalInput").ap(),
        "prev_w": nc.dram_tensor("prev_w", [B, N], f32, kind="ExternalInput").ap(),
        "g": nc.dram_tensor("g", [B, 1], f32, kind="ExternalInput").ap(),
        "s": nc.dram_tensor("s", [B, 3], f32, kind="ExternalInput").ap(),
        "gamma": nc.dram_tensor("gamma", [B, 1], f32, kind="ExternalInput").ap(),
    }
    out_ap = nc.dram_tensor("out", [B, N], f32, kind="ExternalOutput").ap()
    _build_body(nc, out_ap, ins)
    nc.finalize()
    _NC_CACHE["nc"] = nc
    return nc


def _shard_inputs(inputs):
    arrs = {
        name: np.ascontiguousarray(np.asarray(inputs[name], dtype=np.float32))
        for name in ("memory", "k", "beta", "prev_w", "g", "s", "gamma")
    }
    in_maps = []
    for c in range(NCORES):
        sl = slice(c * B, (c + 1) * B)
        in_maps.append({name: np.ascontiguousarray(a[sl]) for name, a in arrs.items()})
    return in_maps


def run(inputs, trace=False):
    from concourse.bass_utils import run_bass_kernel_spmd

    nc = _get_nc()
    in_maps = _shard_inputs(inputs)
    res = run_bass_kernel_spmd(
        nc, in_maps, core_ids=list(range(NCORES)), trace=trace,
        **({"trace_cores": [0]} if trace else {}),
    )
    out = np.concatenate([r["out"] for r in res.results], axis=0)
    return out, res


def kernel(**inputs):
    out, _ = run(inputs, trace=False)
    return out


# revision 50
# speedup vs baseline: 1.4576x; 1.0186x over previous
"""NTM addressing head (nn_HeadBase) Trainium2 Bass kernel.

Full-input contract: kernel(**inputs) takes the unsharded [256, ...] arrays,
shards batch-dim across 8 NeuronCores (pure data parallel), runs one SPMD Bass
program per core, and gathers the full [256, 4096] output.

Per-core layout (B=32 batches, N=4096, M=64):
  memory[b] is streamed as [128, CB*2048] SBUF tiles (CB=4 batches/chunk)
  with n = p*32 + r (partition p, free = (b, r, m)); 8 KB contiguous per
  partition per batch.

  Phase A per chunk: the mem*k multiply is split GpSimd (2.5 batches) / DVE
  (1.5 batches), both writing an fp16 product tile; ACT squares mem into an
  fp16 tile.  The m=64 reductions run as fp16 tensor-tensor halving trees
  (64->32->16->8, 2x DVE rate) finished by a fp32-out native reduce (8->1).
  Emission is software-pipelined: chunk c's mults are emitted before chunk
  c-1's trees so the in-order DVE queue reaches the mult (and frees the mem
  buffer for DMA c+2) without waiting behind tree work.
  Raw k is broadcast to all partitions immediately after its DMA; the
  beta/||k|| scale is applied as one extra phase-B multiply instead of
  delaying the broadcast.

  Phase B (all batches fused as [128, 1024] f32 tiles): a = beta/||k|| *
  dot / sqrt(ssq) (Newton-reciprocal on DVE; exact InstReciprocal costs
  6.5us), softmax (no max-subtract: |a|<1), gated interpolation, 3-tap
  circular shift via shifted APs; the +-1 partition carries go through two
  128x128 circular-shift matmuls on the idle TensorEngine.  pow via exp/ln
  with activation-table preloads hidden behind DVE work.  Per-batch scalars
  are broadcast to [128, B] via K=1 ones-matmuls; PSUM evacuation on DVE.
"""

import numpy as np

B_FULL, N, M = 256, 4096, 64
NCORES = 8
B = B_FULL // NCORES   # 32 batches per core
P = 128                # SBUF partitions
R = N // P             # 32 rows per partition; n = p*R + r

_NC_CACHE = {}


def _build_body(nc, out_ap, ins):
    """Emit the kernel IR. ins: dict name->AP of DRAM inputs, out_ap: DRAM out."""
    from contextlib import ExitStack

    import concourse.bass as bass
    import concourse.tile as tile
    from concourse import mybir

    f32 = mybir.dt.float32
    f16 = mybir.dt.float16
    Alu = mybir.AluOpType
    Act = mybir.ActivationFunctionType
    Ax = mybir.AxisListType
    AP = bass.AP

    mem_ap = ins["memory"]   # [B, N, M]
    k_ap = ins["k"]          # [B, M]
    beta_ap = ins["beta"]    # [B, 1]
    pw_ap = ins["prev_w"]    # [B, N]
    g_ap = ins["g"]          # [B, 1]
    s_ap = ins["s"]          # [B, 3]
    gam_ap = ins["gamma"]    # [B, 1]

    def bcast_inner(ap2d, n):
        # [P, C] -> [P, C, n] with 0-stride inner dim
        return AP(ap2d.tensor, ap2d.offset, list(ap2d.ap) + [[0, n]])

    def row1(ap1d):
        # prepend a unit partition dim to a 1-d AP
        return AP(ap1d.tensor, ap1d.offset, [[0, 1]] + list(ap1d.ap))

    with tile.TileContext(nc) as tc, ExitStack() as ctx:
        singles = ctx.enter_context(tc.tile_pool(name="singles", bufs=1))
        mem_pool = ctx.enter_context(tc.tile_pool(name="mem", bufs=2))
        mth_pool = ctx.enter_context(tc.tile_pool(name="mth", bufs=2))
        pr_pool = ctx.enter_context(tc.tile_pool(name="pr", bufs=2))
        sq_pool = ctx.enter_context(tc.tile_pool(name="sq", bufs=2))
        h_pool = ctx.enter_context(tc.tile_pool(name="h", bufs=1))
        big = ctx.enter_context(tc.tile_pool(name="big", bufs=1))
        ps = ctx.enter_context(tc.tile_pool(name="ps", bufs=2, space="PSUM"))
        ps_big = ctx.enter_context(tc.tile_pool(name="psbig", bufs=1, space="PSUM"))

        # ---- setup ----
        # Small-input DMAs issued from idle engine sequencers (scalar/gpsimd)
        # so the Sync engine's in-order queue is free to start generating the
        # big memory-chunk descriptors immediately.
        ones_col = singles.tile([P, 1], f32, tag="ones_col")
        nc.vector.memset(ones_col, 1.0)
        ones_row = singles.tile([1, P], f32, tag="ones_row")
        nc.vector.memset(ones_row, 1.0)

        # All small-input DMAs on GpSimd's sequencer (idle, and GP activity
        # at t<15us predates any DVE 2x op): ACT's queue stays free for the
        # kb copy + chunk-0 converts, Sync's for chunk descriptors.
        # k borrows a rotating memory-chunk slot (it is dead before chunk 2
        # rotates onto this slot; Tile's WAR tracking enforces the ordering)
        k_host = mem_pool.tile([P, 4 * R * M], f32, tag="mt")
        k_row = k_host[0:1, 0 : B * M]
        nc.gpsimd.dma_start(out=k_row, in_=row1(k_ap.rearrange("b m -> (b m)")))
        b_row = singles.tile([1, B], f32, tag="b_row")
        nc.gpsimd.dma_start(out=b_row, in_=row1(beta_ap.rearrange("b one -> (b one)")))
        g_row = singles.tile([1, B], f32, tag="g_row")
        nc.gpsimd.dma_start(out=g_row, in_=row1(g_ap.rearrange("b one -> (b one)")))
        gm_row = singles.tile([1, B], f32, tag="gm_row")
        nc.gpsimd.dma_start(out=gm_row, in_=row1(gam_ap.rearrange("b one -> (b one)")))
        s_row = singles.tile([1, 3 * B], f32, tag="s_row")
        nc.gpsimd.dma_start(out=s_row, in_=row1(s_ap.rearrange("b i -> (b i)")))
        # prev_w big tile [P, B*R] in one permuted-AP DMA (128B inner runs)
        pw = big.tile([P, B * R], f32, tag="pw")
        nc.gpsimd.dma_start(
            out=pw.rearrange("p (b r) -> p b r", r=R),
            in_=pw_ap.rearrange("b (p r) -> p b r", r=R),
        )
        # s_i as [1, B] strided views (stride 3)
        s_perm = s_row.rearrange("p (b i) -> p i b", i=3)
        s_v = [s_perm[:, i, :] for i in range(3)]

        # RAW k broadcast to all partitions ASAP: kb[p, b*M+m] = k[b, m].
        # kb is fp16 so the phase-A multiply runs in the DVE's 2x mode.
        kb_psum = ps_big.tile([P, B * M], f32, tag="kb_psum")
        for j in range(0, B * M, 512):
            nc.tensor.matmul(
                kb_psum[:, j : j + 512], ones_row, k_row[:, j : j + 512],
                start=True, stop=True,
            )
        kb = singles.tile([P, B * M], f16, tag="kb")
        nc.scalar.copy(out=kb, in_=kb_psum)

        # ---- phase A: stream memory in CB-batch chunks ----
        # NOTE: all remaining phase-B setup (bk chain, scalar broadcasts, pwo,
        # shift matrices) is emitted AFTER the last chunk's stream ops so it
        # cannot block the in-order engine queues during phase A.
        CB = 4          # batches per chunk
        NCH = B // CB   # 8 chunks
        dot = big.tile([P, B * R], f32, tag="dot")
        ssq = big.tile([P, B * R], f32, tag="ssq")

        # GpSimd is deliberately UNUSED in phase A: any Q7 activity stalls
        # DVE double-pumped (2x) ops completely (observed on HW), and the
        # whole phase-A pipeline below runs the DVE in 2x mode.  ACT instead
        # converts the stream to fp16 (enabling the 2x multiply) and squares.
        def emit_stream(c):
            """DMA + f16 convert + mult + square for chunk c -> (pr, sq)."""
            b0 = c * CB
            mt = mem_pool.tile([P, CB * R * M], f32, tag="mt")
            mtb = mt.rearrange("p (b f) -> p b f", b=CB)
            # two half-chunk DMAs for finer arrival granularity
            nc.sync.dma_start(
                out=mtb[:, 0:2],
                in_=mem_ap[b0 : b0 + 2].rearrange("b (p r) m -> p b (r m)", p=P),
            )
            nc.sync.dma_start(
                out=mtb[:, 2:4],
                in_=mem_ap[b0 + 2 : b0 + 4].rearrange(
                    "b (p r) m -> p b (r m)", p=P),
            )
            # ACT: fp16 copy of the stream (feeds the 2x multiply), then
            # squares straight from the f32 stream (fp16 out)
            mth = mth_pool.tile([P, CB * R * M], f16, tag="mth")
            mthb = mth.rearrange("p (b f) -> p b f", b=CB)
            nc.scalar.copy(out=mthb[:, 0:2], in_=mtb[:, 0:2])
            nc.scalar.copy(out=mthb[:, 2:4], in_=mtb[:, 2:4])
            sq = sq_pool.tile([P, CB * R * M], f16, tag="sq")
            sqb = sq.rearrange("p (b f) -> p b f", b=CB)
            nc.scalar.square(out=sqb[:, 0:2], in_=mtb[:, 0:2])
            nc.scalar.square(out=sqb[:, 2:4], in_=mtb[:, 2:4])
            # DVE: f16 multiply at 2x
            pr = pr_pool.tile([P, CB * R * M], f16, tag="pr")
            pr4 = pr.rearrange("p (b r m) -> p b r m", b=CB, m=M)
            mth4 = mth.rearrange("p (b r m) -> p b r m", b=CB, m=M)
            kbc = kb[:, b0 * M : (b0 + CB) * M]  # [P, CB*M]
            kb4 = AP(kbc.tensor, kbc.offset, [kbc.ap[0], [M, CB], [0, R], [1, M]])
            nc.vector.tensor_tensor(
                out=pr4, in0=mth4, in1=kb4, op=Alu.mult
            )
            return pr, sq

        def emit_trees(c, pr, sq):
            """fp16 halving trees (2x DVE) + f32-out final reduce for chunk c."""
            b0 = c * CB
            G = CB * R  # 128 groups of 64
            for src, dst in ((pr, dot), (sq, ssq)):
                v64 = src.rearrange("p (g m) -> p g m", m=64)
                h1 = h_pool.tile([P, G * 32], f16, tag="h1")
                h1v = h1.rearrange("p (g m) -> p g m", m=32)
                nc.vector.tensor_add(
                    out=h1v, in0=v64[:, :, 0:32], in1=v64[:, :, 32:64]
                )
                h2 = h_pool.tile([P, G * 16], f16, tag="h2")
                h2v = h2.rearrange("p (g m) -> p g m", m=16)
                nc.vector.tensor_add(
                    out=h2v, in0=h1v[:, :, 0:16], in1=h1v[:, :, 16:32]
                )
                h3 = h_pool.tile([P, G * 8], f16, tag="h3")
                h3v = h3.rearrange("p (g m) -> p g m", m=8)
                nc.vector.tensor_add(
                    out=h3v, in0=h2v[:, :, 0:8], in1=h2v[:, :, 8:16]
                )
                nc.vector.tensor_reduce(
                    out=dst[:, b0 * R : (b0 + CB) * R].rearrange(
                        "p (b r) -> p b r", b=CB),
                    in_=h3v.rearrange("p (b r) m -> p (b r) m", b=CB),
                    axis=Ax.X, op=Alu.add,
                )

        def emit_stream0():
            """Chunk 0 at per-batch granularity: halves the fill latency
            (the first multiply starts once two batches have landed and
            been converted, instead of waiting for the full 4-batch tile)."""
            mt = mem_pool.tile([P, CB * R * M], f32, tag="mt")
            mtb = mt.rearrange("p (b f) -> p b f", b=CB)
            mth = mth_pool.tile([P, CB * R * M], f16, tag="mth")
            mthb = mth.rearrange("p (b f) -> p b f", b=CB)
            sq = sq_pool.tile([P, CB * R * M], f16, tag="sq")
            sqb = sq.rearrange("p (b f) -> p b f", b=CB)
            for b in range(CB):
                nc.sync.dma_start(
                    out=mtb[:, b : b + 1],
                    in_=mem_ap[b : b + 1].rearrange(
                        "b (p r) m -> p b (r m)", p=P),
                )
                nc.scalar.copy(out=mthb[:, b : b + 1], in_=mtb[:, b : b + 1])
            pr = pr_pool.tile([P, CB * R * M], f16, tag="pr")
            pr4 = pr.rearrange("p (b r m) -> p b r m", b=CB, m=M)
            mth4 = mth.rearrange("p (b r m) -> p b r m", b=CB, m=M)
            kbc = kb[:, 0 : CB * M]
            for h in range(2):
                kb4 = AP(
                    kbc.tensor, kbc.offset + h * 2 * M,
                    [kbc.ap[0], [M, 2], [0, R], [1, M]],
                )
                nc.vector.tensor_tensor(
                    out=pr4[:, 2 * h : 2 * h + 2],
                    in0=mth4[:, 2 * h : 2 * h + 2], in1=kb4, op=Alu.mult,
                )
            nc.scalar.square(out=sqb[:, 0:2], in_=mtb[:, 0:2])
            nc.scalar.square(out=sqb[:, 2:4], in_=mtb[:, 2:4])
            return pr, sq

        st0 = emit_stream0()

        # ---- phase-B setup, emitted while the engines wait for chunk 0's
        # DMA (~7us of DVE idle): fills the pipeline-fill window for free ----
        # bk = beta / ||k||; k is squared in place (k_row is dead once the
        # kb broadcast matmuls have read it -- Tile WAR tracking orders this)
        nc.vector.tensor_mul(k_row, k_row, k_row)
        ks_row = singles.tile([1, B], f32, tag="ks_row")
        nc.vector.tensor_reduce(
            out=ks_row, in_=k_row.rearrange("p (b m) -> p b m", m=M),
            axis=Ax.X, op=Alu.add,
        )
        # this Sqrt also preloads the table for the big ssq sqrt below
        kn_row = singles.tile([1, B], f32, tag="kn_row")
        nc.scalar.activation(out=kn_row, in_=ks_row, func=Act.Sqrt)
        rk_row = singles.tile([1, B], f32, tag="rk_row")
        nc.vector.reciprocal(out=rk_row, in_=kn_row)
        bk_row = singles.tile([1, B], f32, tag="bk_row")
        nc.vector.tensor_mul(bk_row, b_row, rk_row)
        # omg = 1 - g
        omg_row = singles.tile([1, B], f32, tag="omg_row")
        nc.vector.tensor_scalar(
            out=omg_row, in0=g_row, scalar1=-1.0, scalar2=1.0,
            op0=Alu.mult, op1=Alu.add,
        )
        # broadcast round 1: [bk, omg, s0, s1, s2, gamma] -> [P, 6*B]
        NSC = 6
        asm1 = singles.tile([1, NSC * B], f32, tag="asm1")
        for i, src in enumerate([bk_row, omg_row, s_v[0], s_v[1], s_v[2], gm_row]):
            nc.vector.tensor_copy(asm1[:, i * B : (i + 1) * B], src)
        bc1_ps = ps.tile([P, NSC * B], f32, tag="mm")
        nc.tensor.matmul(bc1_ps, ones_row, asm1, start=True, stop=True)
        BC1 = singles.tile([P, NSC * B], f32, tag="BC1")
        nc.vector.tensor_copy(BC1, bc1_ps)
        BK = BC1[:, 0 * B : 1 * B]
        OMG = BC1[:, 1 * B : 2 * B]
        S0 = BC1[:, 2 * B : 3 * B]
        S1 = BC1[:, 3 * B : 4 * B]
        S2 = BC1[:, 4 * B : 5 * B]
        GAM = BC1[:, 5 * B : 6 * B]
        # pwo = prev_w * (1 - g), in fp16 for the 2x gating chain
        pwo = singles.tile([P, B * R], f16, tag="pwo")
        nc.vector.tensor_mul(
            pwo.rearrange("p (b r) -> p b r", r=R),
            pw.rearrange("p (b r) -> p b r", r=R),
            bcast_inner(OMG, R),
        )
        # fp16 copies of the shift taps for the 2x shift chain
        Sh = singles.tile([P, 3 * B], f16, tag="Sh")
        nc.vector.tensor_copy(Sh, BC1[:, 2 * B : 5 * B])
        S0h = Sh[:, 0 * B : 1 * B]
        S1h = Sh[:, 1 * B : 2 * B]
        S2h = Sh[:, 2 * B : 3 * B]

        # circular +-1 partition shift matrices for the conv carries (on the
        # DVE, still inside the chunk-0 fill window; GpSimd must stay idle):
        # SD[p, q] = 1 iff q == (p+1) mod P ; SU[p, q] = 1 iff q == (p-1) mod P
        ones_sq = singles.tile([P, P], f16, tag="ones_sq")
        nc.vector.memset(ones_sq, 1.0)
        sd_t = singles.tile([P, P], f16, tag="sd_t")
        nc.gpsimd.affine_select(
            out=sd_t, in_=ones_sq, pattern=[[1, P]], compare_op=Alu.is_equal,
            fill=0.0, base=-1, channel_multiplier=-1,
        )
        SD = singles.tile([P, P], f16, tag="SD")
        nc.gpsimd.affine_select(
            out=SD, in_=sd_t, pattern=[[1, P]], compare_op=Alu.not_equal,
            fill=1.0, base=P - 1, channel_multiplier=-1,
        )
        su_t = singles.tile([P, P], f16, tag="sd_t")
        nc.gpsimd.affine_select(
            out=su_t, in_=ones_sq, pattern=[[1, P]], compare_op=Alu.is_equal,
            fill=0.0, base=1, channel_multiplier=-1,
        )
        SU = singles.tile([P, P], f16, tag="SU")
        nc.gpsimd.affine_select(
            out=SU, in_=su_t, pattern=[[1, P]], compare_op=Alu.not_equal,
            fill=1.0, base=-(P - 1), channel_multiplier=-1,
        )

        emit_trees(0, *st0)
        for c in range(1, NCH):
            st = emit_stream(c)
            emit_trees(c, *st)

        # re-preload the Sqrt table after the last square so the phase-B sqrt
        # doesn't pay the switch (the kn sqrt above loaded it too early)
        dummy = singles.tile([1, 1], f32, tag="dummy")
        nc.scalar.activation(out=dummy, in_=ks_row[:, 0:1], func=Act.Sqrt)

        # ---- phase B ----
        def v3(t):
            return t.rearrange("p (b r) -> p b r", r=R)

        # a = (beta/||k||) * dot / sqrt(ssq)
        # rstd lands in ws (overwritten later by the shift); the Newton
        # scratch bitcasts a dead fp16 product tile.
        nc.scalar.activation(out=ssq, in_=ssq, func=Act.Sqrt)
        # preload the Exp table while DVE runs the reciprocal
        nc.scalar.activation(out=dummy, in_=ks_row[:, 0:1], func=Act.Exp)
        ws = big.tile([P, B * R], f32, tag="ws")
        scr16 = sq_pool.tile([P, CB * R * M], f16, tag="sq")
        nc.vector.reciprocal_approx_fast(out=ws, in_=ssq)
        nc.vector.tensor_mul(dot, dot, ws)
        nc.vector.tensor_mul(v3(dot), v3(dot), bcast_inner(BK, R))

        # fp16 scratch for the gating/shift chain, carved out of the dead
        # fp16 square tile: the DVE runs these element-wise ops at 2x
        # (GpSimd is idle here, so the 2x/Q7 conflict cannot bite).
        e16 = scr16[:, 0 * B * R : 1 * B * R]
        ws16 = scr16[:, 1 * B * R : 2 * B * R]
        ta16 = scr16[:, 2 * B * R : 3 * B * R]
        tb16 = scr16[:, 3 * B * R : 4 * B * R]
        wp16 = scr16[:, 4 * B * R : 5 * B * R]

        # e = exp(a), fp16
        nc.scalar.activation(out=e16, in_=dot, func=Act.Exp)
        # preload the Ln table while DVE runs the softmax/gating chain
        nc.scalar.activation(out=dummy, in_=ks_row[:, 0:1], func=Act.Ln)

        # denom per batch; gd = g/denom
        cs = singles.tile([P, B], f32, tag="cs")
        nc.vector.tensor_reduce(out=cs, in_=v3(e16), axis=Ax.X, op=Alu.add)
        den_ps = ps.tile([1, B], f32, tag="mm")
        nc.tensor.matmul(den_ps, ones_col, cs, start=True, stop=True)
        rden_row = singles.tile([1, B], f32, tag="rden_row")
        nc.vector.reciprocal(out=rden_row, in_=den_ps)
        gd_row = singles.tile([1, B], f32, tag="gd_row")
        nc.vector.tensor_mul(gd_row, rden_row, g_row)
        gd_ps = ps.tile([P, B], f32, tag="mm")
        nc.tensor.matmul(gd_ps, ones_row, gd_row, start=True, stop=True)
        GDh = singles.tile([P, B], f16, tag="GDh")
        nc.vector.tensor_copy(GDh, gd_ps)

        # wg = e*gd + pwo   (in place into e16)
        nc.vector.tensor_mul(v3(e16), v3(e16), bcast_inner(GDh, R))
        nc.vector.tensor_add(out=e16, in0=e16, in1=pwo)

        # circular 3-tap shift: ws[n] = s1*wg[n] + s0*wg[n-1] + s2*wg[n+1]
        wg3, ws3h, ta3, tb3 = v3(e16), v3(ws16), v3(ta16), v3(tb16)
        nc.vector.tensor_mul(ta3, wg3, bcast_inner(S0h, R))
        nc.vector.tensor_mul(tb3, wg3, bcast_inner(S2h, R))
        nc.vector.tensor_mul(ws3h, wg3, bcast_inner(S1h, R))
        # partition carries via circular-shift matmuls on the TensorEngine
        # (issued as soon as ta/tb are ready, overlapping the shifted adds):
        # dn[q, b] = ta[(q-1) mod P, b, R-1];  up[q, b] = tb[(q+1) mod P, b, 0]
        ta_col = AP(ta16.tensor, ta16.offset + (R - 1), [ta16.ap[0], [R, B]])
        tb_col = AP(tb16.tensor, tb16.offset, [tb16.ap[0], [R, B]])
        dn_ps = ps.tile([P, B], f32, tag="mm")
        nc.tensor.matmul(dn_ps, SD, ta_col, start=True, stop=True)
        up_ps = ps.tile([P, B], f32, tag="mm")
        nc.tensor.matmul(up_ps, SU, tb_col, start=True, stop=True)
        nc.vector.tensor_add(
            out=ws3h[:, :, 1:R], in0=ws3h[:, :, 1:R], in1=ta3[:, :, 0 : R - 1]
        )
        nc.vector.tensor_add(
            out=ws3h[:, :, 0 : R - 1], in0=ws3h[:, :, 0 : R - 1],
            in1=tb3[:, :, 1:R],
        )
        nc.vector.tensor_add(
            out=ws3h[:, :, 0:1], in0=ws3h[:, :, 0:1], in1=bcast_inner(dn_ps, 1)
        )
        nc.vector.tensor_add(
            out=ws3h[:, :, R - 1 : R], in0=ws3h[:, :, R - 1 : R],
            in1=bcast_inner(up_ps, 1),
        )

        # w_pow = ws ** gamma = exp(gamma * ln(ws))
        nc.scalar.activation(out=ws, in_=ws16, func=Act.Ln)
        # preload the Exp table while DVE runs the gamma multiply
        nc.scalar.activation(out=dummy, in_=ks_row[:, 0:1], func=Act.Exp)
        ws3 = v3(ws)
        nc.vector.tensor_mul(ws3, ws3, bcast_inner(GAM, R))
        nc.scalar.activation(out=wp16, in_=ws, func=Act.Exp)

        # normalize: out = w_pow / (sum + 1e-16)
        cs2 = singles.tile([P, B], f32, tag="cs2")
        nc.vector.tensor_reduce(out=cs2, in_=v3(wp16), axis=Ax.X, op=Alu.add)
        d2_ps = ps.tile([1, B], f32, tag="mm")
        nc.tensor.matmul(d2_ps, ones_col, cs2, start=True, stop=True)
        d2_row = singles.tile([1, B], f32, tag="d2_row")
        nc.vector.tensor_scalar_add(out=d2_row, in0=d2_ps, scalar1=1e-16)
        rd2_row = singles.tile([1, B], f32, tag="rd2_row")
        nc.vector.reciprocal(out=rd2_row, in_=d2_row)
        rd2_ps = ps.tile([P, B], f32, tag="mm")
        nc.tensor.matmul(rd2_ps, ones_row, rd2_row, start=True, stop=True)
        RD2 = singles.tile([P, B], f32, tag="RD2")
        nc.vector.tensor_copy(RD2, rd2_ps)
        nc.vector.tensor_mul(ws3, v3(wp16), bcast_inner(RD2, R))

        nc.sync.dma_start(
            out=out_ap.rearrange("b (p r) -> p b r", r=R),
            in_=ws.rearrange("p (b r) -> p b r", r=R),
        )


def _get_nc():
    if "nc" in _NC_CACHE:
        return _NC_CACHE["nc"]
    from concourse import bacc, mybir

    f32 = mybir.dt.float32
    nc = bacc.Bacc("TRN2", debug=False, num_devices=NCORES)
    ins = {
        "memory": nc.dram_tensor("memory", [B, N, M], f32, kind="ExternalInput").ap(),
        "k": nc.dram_tensor("k", [B, M], f32, kind="ExternalInput").ap(),
        "beta": nc.dram_tensor("beta", [B, 1], f32, kind="ExternalInput").ap(),
        "prev_w": nc.dram_tensor("prev_w", [B, N], f32, kind="ExternalInput").ap(),
        "g": nc.dram_tensor("g", [B, 1], f32, kind="ExternalInput").ap(),
        "s": nc.dram_tensor("s", [B, 3], f32, kind="ExternalInput").ap(),
        "gamma": nc.dram_tensor("gamma", [B, 1], f32, kind="ExternalInput").ap(),
    }
    out_ap = nc.dram_tensor("out", [B, N], f32, kind="ExternalOutput").ap()
    _build_body(nc, out_ap, ins)
    nc.finalize()
    _NC_CACHE["nc"] = nc
    return nc


def _shard_inputs(inputs):
    arrs = {
        name: np.ascontiguousarray(np.asarray(inputs[name], dtype=np.float32))
        for name in ("memory", "k", "beta", "prev_w", "g", "s", "gamma")
    }
    in_maps = []
    for c in range(NCORES):
        sl = slice(c * B, (c + 1) * B)
        in_maps.append({name: np.ascontiguousarray(a[sl]) for name, a in arrs.items()})
    return in_maps


def run(inputs, trace=False):
    from concourse.bass_utils import run_bass_kernel_spmd

    nc = _get_nc()
    in_maps = _shard_inputs(inputs)
    res = run_bass_kernel_spmd(
        nc, in_maps, core_ids=list(range(NCORES)), trace=trace,
        **({"trace_cores": [0]} if trace else {}),
    )
    out = np.concatenate([r["out"] for r in res.results], axis=0)
    return out, res


def kernel(**inputs):
    out, _ = run(inputs, trace=False)
    return out
